# revision 1
# baseline (speedup 1.0000x reference)
"""Masked multi-head attention on 8 TRN2 NeuronCores.

Sharding: core = (batch b, head-group hg). Each core computes the attention
output for one batch element and 4 of the 8 heads (a 256-wide column slice
of E). Rows with mask==0 are dropped host-side before the kernel runs:
masked queries produce all-zero output rows, and masked keys are excluded
from the softmax, so the kernel only processes the ~half of S that is live
(gathered and padded to a multiple of 128).

Input is packed host-side into one bf16/fp32r blob per core and DMA'd in
progressive chunks over both HWDGE rings so the v/k projections start
while later xT groups are still in flight.

Per-core on-chip pipeline (scores in fp32r, PV in bf16, fp32 PSUM):
  qT/kT = W.T @ xT         (E-cols on partitions, S free)
  v     = xT.T @ Wv        (S on partitions, DH free) + ones column
  sT    = kT_chunk.T @ qT  (keys on partitions, queries free)
  att   = exp(sT/8 + pad_bias)                  [ACT, bias masks pad keys]
  hT   += v_aug.T @ att    (accumulates h' and the softmax denominator)
  out   = hT (+den row) DMA'd per head; the host transposes and divides
"""

import os

import numpy as np

import concourse.bacc as bacc
import concourse.tile as tile
from concourse import mybir
from concourse.bass_utils import run_bass_kernel_spmd

BF = mybir.dt.bfloat16
F32 = mybir.dt.float32
MMDT = mybir.dt.float32r  # fp32 storage, full-rate single-pass PE mode

B, S, F, E, H = 4, 2048, 512, 512, 8
DH = 64
NCORES = 8
HPC = 4            # heads per core
CPC = HPC * DH     # output columns per core

LAST_RESULT = None  # BassKernelResults of the most recent run (for test harness)


def _qchunks(SPL):
    # 512-wide chunks plus remainder; boundaries stay 128-aligned for the
    # kc-major xT layout. SPL must be even (fp32r 128-row matmuls).
    out, off = [], 0
    while off < SPL:
        ln = min(512, SPL - off)
        out.append((off, ln))
        off += ln
    return out


def _offsets(SP):
    # W stored v|k|q so the v projection can start earliest; xT stored
    # kc-major so progressive DMAs unlock v-projection chunks as they land
    WV_OFF = 0
    WK_OFF = 1024
    WQ_OFF = 2048
    BK_OFF = 3072
    BV_OFF = BK_OFF + 256
    ONES_OFF = BK_OFF + 512
    ONES2_OFF = ONES_OFF + SP      # [128, HPC] of ones (v_aug denominator cols)
    XT_OFF = ONES2_OFF + HPC
    ETE_OFF = XT_OFF + 4 * SP
    COLS = ETE_OFF + 2 * SP
    return WV_OFF, WK_OFF, WQ_OFF, XT_OFF, ETE_OFF, BK_OFF, BV_OFF, ONES_OFF, ONES2_OFF, COLS


def _build(SP, loop_reps=None, abl="full", SPL=None, pairq=True, has_bias=True):
    if SPL is None:
        SPL = SP
    NKC = SP // 128
    (WV_OFF, WK_OFF, WQ_OFF, XT_OFF, ETE_OFF, BK_OFF, BV_OFF, ONES_OFF,
     ONES2_OFF, COLS) = _offsets(SP)

    nc = bacc.Bacc()
    blob = nc.declare_dram_parameter("blob", [128, COLS], MMDT, isOutput=False)
    miscf = nc.declare_dram_parameter("miscf", [128, NKC + 65], F32, isOutput=False)
    outp = nc.declare_dram_parameter("out", [HPC, 65, SP], F32, isOutput=True)

    with tile.TileContext(nc) as tc:
        with (
            tc.tile_pool(name="sing", bufs=1) as sing,
            tc.tile_pool(name="hsb", bufs=3) as hsb_pool,
            tc.tile_pool(name="attp", bufs=6) as attp,
            tc.tile_pool(name="ps", bufs=2, space="PSUM") as ps,
        ):
            def _body():
                _emit(nc, SP, SPL, NKC, WV_OFF, WK_OFF, WQ_OFF, XT_OFF, ETE_OFF,
                      BK_OFF, BV_OFF, ONES_OFF, ONES2_OFF, COLS, blob, miscf, outp,
                      sing, hsb_pool, attp, ps, abl, pairq, has_bias)

            if loop_reps is None:
                _body()
            else:
                with tc.For_i(0, loop_reps, 1):
                    _body()
    nc.compile()
    return nc


def _xt_moving(bsb, XT_OFF, SP, f, qoff, qlen):
    """Moving-operand APs over the kc-major xT layout for q range [qoff, qoff+qlen)."""
    view = bsb[:, XT_OFF:XT_OFF + 4 * SP].rearrange("p (kc f c) -> p kc f c", f=4, c=128)
    out = []
    kc0, nfull, rem = qoff // 128, qlen // 128, qlen % 128
    if nfull:
        out.append((0, nfull * 128, view[:, kc0:kc0 + nfull, f, :]))
    if rem:
        out.append((nfull * 128, rem, view[:, kc0 + nfull, f, :rem]))
    return out


def _emit(nc, SP, SPL, NKC, WV_OFF, WK_OFF, WQ_OFF, XT_OFF, ETE_OFF, BK_OFF,
          BV_OFF, ONES_OFF, ONES2_OFF, COLS, blob, miscf, outp, sing, hsb_pool,
          attp, ps, abl="full", pairq=True, has_bias=True):
    QCH = _qchunks(SPL)
    NKCL = (SPL + 127) // 128
    # pairs of consecutive q chunks share one 2-bank psum tile / one exp op
    if pairq:
        PAIRS = [QCH[i:i + 2] for i in range(0, len(QCH), 2)]
    else:
        PAIRS = [[c] for c in QCH]
    PW = max(len(p) for p in PAIRS)
    SBUFS = 3 if PW == 2 else 4

    bsb = sing.tile([128, COLS], MMDT)
    msb = sing.tile([128, NKC + 65], F32)
    KG = [(0, min(4, NKC))]
    while KG[-1][1] < NKC:
        KG.append((KG[-1][1], min(KG[-1][1] + 4, NKC)))
    nc.sync.dma_start(out=bsb[:, :WK_OFF], in_=blob[:, :WK_OFF])
    nc.scalar.dma_start(out=msb, in_=miscf[:, :])
    nc.scalar.dma_start(out=bsb[:, WK_OFF:XT_OFF], in_=blob[:, WK_OFF:XT_OFF])
    for gi, (k0, k1) in enumerate(KG):
        eng = nc.sync if gi % 2 == 0 else nc.scalar
        c0, c1 = XT_OFF + k0 * 512, XT_OFF + k1 * 512
        eng.dma_start(out=bsb[:, c0:c1], in_=blob[:, c0:c1])
    nc.scalar.dma_start(out=bsb[:, ETE_OFF:], in_=blob[:, ETE_OFF:])

    qk = sing.tile([128, 4, SP], BF if abl == "bf16sc" else MMDT)  # qT 0-1, kT 2-3
    vall = sing.tile([128, NKC, 65 * HPC], BF)
    scr_d = sing.tile([1, 1], MMDT)
    scr_a = sing.tile([1, 1], F32)

    ones_row = bsb[0:1, ONES_OFF:ONES_OFF + SP]

    # Engine preambles: each engine observes the input DMA lanes via a cheap
    # op so no later instruction needs two fresh semaphore waits (hardware
    # allows one sync wait per instruction; extras cost event-sem splits).
    nc.vector.tensor_copy(scr_d, bsb[0:1, 0:1])
    nc.vector.tensor_copy(scr_d, bsb[0:1, XT_OFF:XT_OFF + 1])
    nc.vector.tensor_copy(scr_d, bsb[0:1, ETE_OFF:ETE_OFF + 1])
    nc.scalar.copy(scr_a, msb[0:1, 0:1])

    # ones columns of v_aug, early so PE's DVE clock covers them
    ones2 = bsb[:, ONES2_OFF:ONES2_OFF + HPC]
    for kc in range(NKC):
        va = vall[:, kc, :].rearrange("p (h c) -> p h c", c=65)
        nc.vector.tensor_copy(va[:, :, 64:65], ones2.rearrange("p (h c) -> p h c", c=1))

    if abl == "dmas":
        return

    # ---- projections, interleaved with the progressive xT DMA groups:
    # group g unlocks v[kc in g] and the k/q chunks whose kc range lies in
    # groups <= g. v first, then k; q last (waits on the ete DMA anyway).
    def v_proj(kc):
        pv = ps.tile([128, PW, 512], F32, tag="s2", bufs=SBUFS, name="pv")
        if has_bias:
            nc.tensor.matmul(pv[:, 0, :256], ones_row[:, 0:128], bsb[0:1, BV_OFF:BV_OFF + 256],
                             start=True, stop=False)
        for f in range(4):
            base = XT_OFF + (kc * 4 + f) * 128
            lhsT = bsb[:, base:base + 128]
            rhs = bsb[:, WV_OFF + f * 256:WV_OFF + (f + 1) * 256]
            nc.tensor.matmul(pv[:, 0, :256], lhsT, rhs,
                             start=(f == 0 and not has_bias), stop=(f == 3))
        va = vall[:, kc, :].rearrange("p (h c) -> p h c", c=65)
        nc.vector.tensor_copy(va[:, :, 0:64], pv[:, 0, :256].rearrange("p (h c) -> p h c", c=64))

    def kq_proj(cc, pair):
        p = ps.tile([128, PW, 512], F32, tag="s2", bufs=SBUFS, name="p")
        for j, (qoff, qlen) in enumerate(pair):
            if cc >= 2 and has_bias:  # k: rank-1 bias init (bk x ones)
                bksl = bsb[0:1, BK_OFF + (cc - 2) * 128:BK_OFF + (cc - 1) * 128]
                nc.tensor.matmul(p[:, j, :qlen], bksl, ones_row[:, qoff:qoff + qlen],
                                 start=True, stop=False)
            nparts = len(_xt_moving(bsb, XT_OFF, SP, 0, qoff, qlen))
            for pi in range(nparts):
                for f in range(4):
                    if cc < 2:
                        woff = WQ_OFF + f * 256 + cc * 128
                    else:
                        woff = WK_OFF + f * 256 + (cc - 2) * 128
                    lhsT = bsb[:, woff:woff + 128]
                    loff, llen, ap = _xt_moving(bsb, XT_OFF, SP, f, qoff, qlen)[pi]
                    nc.tensor.matmul(p[:, j, loff:loff + llen], lhsT, ap,
                                     start=(f == 0 and (cc < 2 or not has_bias)),
                                     stop=(f == 3))
        qoff0 = pair[0][0]
        width = (512 + pair[1][1]) if len(pair) == 2 else pair[0][1]
        pview = p[:].rearrange("p a b -> p (a b)")[:, :width]
        if cc < 2:  # q: add etype_emb (includes bq)
            ete_sl = bsb[:, ETE_OFF + cc * SP + qoff0:ETE_OFF + cc * SP + qoff0 + width]
            nc.vector.tensor_add(qk[:, cc, qoff0:qoff0 + width], pview, ete_sl)
        else:
            nc.vector.tensor_copy(qk[:, cc, qoff0:qoff0 + width], pview)

    QCH_K = _qchunks(SP)
    if pairq:
        PAIRS_K = [QCH_K[i:i + 2] for i in range(0, len(QCH_K), 2)]
    else:
        PAIRS_K = [[c] for c in QCH_K]

    def pair_group(pair):  # last xT group this pair's kc range touches
        qoff, qlen = pair[-1]
        return ((qoff + qlen - 1) // 128) // 4

    for gi, (k0, k1) in enumerate(KG):
        for kc in range(k0, k1):
            v_proj(kc)
        for cc in (2, 3):
            for pair in PAIRS_K:
                if pair_group(pair) == gi:
                    kq_proj(cc, pair)
    for cc in (0, 1):
        for pair in PAIRS:
            kq_proj(cc, pair)

    if abl == "proj":
        return

    # ---- attention, software-pipelined: the scores matmuls for step i+1 are
    # emitted before step i's PV matmuls, so the in-order PE never sits
    # behind the exp running on ACT.
    def scores_mm(step, sp_tile):
        h, ip, kc = step
        cbase = (h % 2) * 64
        for j, (qoff, qlen) in enumerate(PAIRS[ip]):
            lhsT = qk[cbase:cbase + 64, 2 + h // 2, kc * 128:(kc + 1) * 128]
            rhs = qk[cbase:cbase + 64, h // 2, qoff:qoff + qlen]
            nc.tensor.matmul(sp_tile[:, j, :qlen], lhsT, rhs, start=True, stop=True)

    steps = [(h, ip, kc)
             for h in range(HPC) for ip in range(len(PAIRS)) for kc in range(NKC)]
    hts = None
    hp = {}
    DEPTH = 2
    sp_q = []
    for d in range(min(DEPTH, len(steps))):
        t = ps.tile([128, PW, 512], F32, tag="s2", bufs=SBUFS, name="sp_t")
        scores_mm(steps[d], t)
        sp_q.append(t)
    for i, step in enumerate(steps):
        h, ip, kc = step
        pair = PAIRS[ip]
        width = (512 + pair[1][1]) if len(pair) == 2 else pair[0][1]
        sp_cur = sp_q.pop(0)
        if i + DEPTH < len(steps):
            sp_next = ps.tile([128, PW, 512], F32, tag="s2", bufs=SBUFS, name="sp_t")
            scores_mm(steps[i + DEPTH], sp_next)
            sp_q.append(sp_next)
        att = attp.tile([128, PW, 512], BF, tag="att")
        nc.scalar.activation(att[:].rearrange("p a b -> p (a b)")[:, :width],
                             sp_cur[:].rearrange("p a b -> p (a b)")[:, :width],
                             mybir.ActivationFunctionType.Exp,
                             bias=msb[:, kc:kc + 1], scale=0.125)
        if abl != "nopv":
            if kc == 0 and ip == 0:
                hts = hsb_pool.tile([65, NKCL * 128], F32, tag="hts")
            for j, (qoff, qlen) in enumerate(pair):
                if kc == 0:
                    hp[j] = ps.tile([65, 512], F32, tag="h", name="hp")
                nc.tensor.matmul(hp[j][:, :qlen], vall[:, kc, h * 65:(h + 1) * 65],
                                 att[:, j, :qlen], start=(kc == 0), stop=(kc == NKC - 1))
            if kc == NKC - 1:
                for j, (qoff, qlen) in enumerate(pair):
                    nc.vector.tensor_copy(hts[:, qoff:qoff + qlen], hp[j][:, :qlen])
                if ip == len(PAIRS) - 1:  # head done: ship hT (+den row); the
                    # host does the [64, q] -> [q, 64] transpose and divide
                    nc.sync.dma_start(out=outp[h, :, :NKCL * 128], in_=hts[:])




def _prep_core(core, SP, x, etype_emb, mask, Wq, bq, Wk, bk, Wv, bv):
    NKC = SP // 128
    (WV_OFF, WK_OFF, WQ_OFF, XT_OFF, ETE_OFF, BK_OFF, BV_OFF, ONES_OFF,
     ONES2_OFF, COLS) = _offsets(SP)
    b, hg = core // 2, core % 2
    c0 = hg * CPC
    idx = np.where(mask[b] == 1)[0]
    Su = len(idx)

    blob = np.zeros((128, COLS), np.float32)
    xs = np.zeros((SP, F), np.float32)
    xs[:Su] = x[b][idx]
    xT = xs.T
    xtb = xT.reshape(4, 128, NKC, 128).transpose(1, 2, 0, 3).reshape(128, NKC * 512)
    blob[:, XT_OFF:XT_OFF + 4 * SP] = xtb
    for f in range(4):
        blob[:, WV_OFF + f * 256:WV_OFF + (f + 1) * 256] = Wv[f * 128:(f + 1) * 128, c0:c0 + CPC]
        blob[:, WK_OFF + f * 256:WK_OFF + (f + 1) * 256] = Wk[f * 128:(f + 1) * 128, c0:c0 + CPC]
        blob[:, WQ_OFF + f * 256:WQ_OFF + (f + 1) * 256] = Wq[f * 128:(f + 1) * 128, c0:c0 + CPC]
    et = np.zeros((SP, CPC), np.float32)
    et[:Su] = etype_emb[b][idx][:, c0:c0 + CPC] + bq[c0:c0 + CPC]
    etT = et.T
    blob[:, ETE_OFF:ETE_OFF + SP] = etT[:128]
    blob[:, ETE_OFF + SP:ETE_OFF + 2 * SP] = etT[128:]
    blob[0, BK_OFF:BK_OFF + CPC] = bk[c0:c0 + CPC]
    blob[0, BV_OFF:BV_OFF + CPC] = bv[c0:c0 + CPC]
    blob[0, ONES_OFF:ONES_OFF + SP] = 1.0
    blob[:, ONES2_OFF:ONES2_OFF + HPC] = 1.0

    miscf = np.zeros((128, NKC + 65), np.float32)
    pos = np.arange(128)[:, None] + 128 * np.arange(NKC)[None, :]
    miscf[:, :NKC] = np.where(pos < Su, 0.0, -30000.0)
    miscf[:65, NKC:NKC + 65] = np.eye(65, dtype=np.float32)

    return {"blob": blob, "miscf": miscf}, idx


def kernel(x, etype_emb, mask, Wq, bq, Wk, bk, Wv, bv):
    global LAST_RESULT
    x = np.asarray(x, np.float32)
    etype_emb = np.asarray(etype_emb, np.float32)
    mask = np.asarray(mask)
    Wq, bq = np.asarray(Wq, np.float32), np.asarray(bq, np.float32)
    Wk, bk = np.asarray(Wk, np.float32), np.asarray(bk, np.float32)
    Wv, bv = np.asarray(Wv, np.float32), np.asarray(bv, np.float32)

    counts = [int((mask[b] == 1).sum()) for b in range(B)]
    SPL = max(2, max(counts))
    SPL += SPL % 2  # fp32r matmuls with 128 contraction rows need even N
    SP = max(128, ((SPL + 127) // 128) * 128)

    has_bias = bool(np.any(bk) or np.any(bv))
    nc = _build(SP, SPL=SPL, has_bias=has_bias)
    in_maps, idxs = [], []
    for core in range(NCORES):
        m, idx = _prep_core(core, SP, x, etype_emb, mask, Wq, bq, Wk, bk, Wv, bv)
        in_maps.append(m)
        idxs.append(idx)

    # The NTFF trace path needs antenv.axon_hooks, which this container does
    # not ship; make sure a stray BASS_TRACE=1 cannot route us into it.
    os.environ.setdefault("BASS_NEVER_TRACE", "1")
    res = run_bass_kernel_spmd(nc, in_maps, list(range(NCORES)))
    LAST_RESULT = res

    out = np.zeros((B, S, E), np.float32)
    for core in range(NCORES):
        b, hg = core // 2, core % 2
        idx = idxs[core]
        if not len(idx):
            continue
        shard = res.results[core]["out"]  # [HPC, 65, SP]: hT rows + denominator
        for h in range(HPC):
            num = shard[h, :64, :len(idx)]
            den = shard[h, 64, :len(idx)]
            out[b][idx, hg * CPC + h * 64:hg * CPC + (h + 1) * 64] = (num / den).T
    return out



# revision 9
# speedup vs baseline: 1.2196x; 1.2196x over previous
"""Masked multi-head attention on 8 TRN2 NeuronCores.

Sharding: core = (batch b, head-group hg). Each core computes the attention
output for one batch element and 4 of the 8 heads (a 256-wide column slice
of E). Rows with mask==0 are dropped host-side before the kernel runs:
masked queries produce all-zero output rows, and masked keys are excluded
via a zeroed "ones" column in the value matrix (their exp(score) is finite
garbage that multiplies a zero v row and a zero denominator weight), so the
kernel only processes the ~half of S that is live (gathered and padded to a
multiple of 128).

Biases never enter the kernel: bq folds into etype_emb host-side, q.bk is
constant per query so it cancels in softmax exactly, and bv satisfies
h = num/den + bv, added host-side.

Per-core on-chip pipeline (fp32r matmuls, exp on ACT, PV in bf16):
  qT/kT = W.T @ xT          (E-cols on partitions, S free)
  v     = xT.T @ Wv         (S on partitions, DH free) + liveness column
  per step (head-pair hp, q-chunk qc, key-chunk kc):
    sT[128k, 2 heads x 512q] = kT.T @ qT   (two matmuls, one psum tile)
    att = exp(sT/8)                        (ONE wide ACT instruction)
    hT[head] += v_aug.T @ att              (accumulates h' and denominator)
  out = hts rows per head DMA'd once; the host transposes and divides.

Projection matmuls are interleaved into the ACT-bound attention phase as PE
filler so the tensor engine stays continuously busy (HAM stays at 2.4GHz).
"""

import os

import numpy as np

import concourse.bacc as bacc
import concourse.tile as tile
from concourse import mybir
from concourse.bass_utils import run_bass_kernel_spmd

BF = mybir.dt.bfloat16
F32 = mybir.dt.float32
MMDT = mybir.dt.float32r  # fp32 storage, full-rate single-pass PE mode

B, S, F, E, H = 4, 2048, 512, 512, 8
DH = 64
NCORES = 8
HPC = 4            # heads per core
CPC = HPC * DH     # output columns per core

LAST_RESULT = None  # BassKernelResults of the most recent run (for test harness)


def _offsets(SP):
    # W stored v|k|q; xT stored kc-major so progressive DMAs unlock
    # projection chunks as they land
    NKC = SP // 128
    WV_OFF = 0
    WK_OFF = 1024
    WQ_OFF = 2048
    BK_OFF = 3072
    BV_OFF = BK_OFF + 256
    ONES_OFF = BK_OFF + 512
    LIVE2_OFF = ONES_OFF + SP      # [128, NKC*HPC] liveness (v_aug den cols)
    XT_OFF = LIVE2_OFF + NKC * HPC
    ETE_OFF = XT_OFF + 4 * SP
    COLS = ETE_OFF + 2 * SP
    return WV_OFF, WK_OFF, WQ_OFF, XT_OFF, ETE_OFF, BK_OFF, BV_OFF, ONES_OFF, LIVE2_OFF, COLS


def _kchunks(total):
    """128-aligned chunks, each <=512 and (when possible) >=256 so fp32r
    matmuls run at full rate."""
    out, off = [], 0
    while total - off > 512 + 255:
        out.append((off, 512))
        off += 512
    rem = total - off
    if rem > 512:
        a = max(256, (rem // 2 // 128) * 128)
        out.append((off, a))
        out.append((off + a, rem - a))
    elif rem:
        out.append((off, rem))
    return out


def _build(SP, loop_reps=None, abl="full", SPL=None, has_bias=True):
    if SPL is None:
        SPL = SP
    NKC = SP // 128
    (WV_OFF, WK_OFF, WQ_OFF, XT_OFF, ETE_OFF, BK_OFF, BV_OFF, ONES_OFF,
     LIVE2_OFF, COLS) = _offsets(SP)

    nc = bacc.Bacc()
    blob = nc.declare_dram_parameter("blob", [128, COLS], MMDT, isOutput=False)
    outp = nc.declare_dram_parameter("out", [HPC, 65, SP], F32, isOutput=True)

    with tile.TileContext(nc) as tc:
        with (
            tc.tile_pool(name="sing", bufs=1) as sing,
            tc.tile_pool(name="hsb", bufs=4) as hsb_pool,
            tc.tile_pool(name="attp", bufs=4) as attp,
            tc.tile_pool(name="ps", bufs=2, space="PSUM") as ps,
        ):
            def _body():
                _emit2(nc, SP, SPL, NKC, WV_OFF, WK_OFF, WQ_OFF, XT_OFF,
                       ETE_OFF, LIVE2_OFF, COLS, blob, outp, sing, hsb_pool,
                       attp, ps, abl)

            if loop_reps is None:
                _body()
            else:
                with tc.For_i(0, loop_reps, 1):
                    _body()
    nc.compile()
    return nc


def _xt_moving(bsb, XT_OFF, SP, f, qoff, qlen):
    """Moving-operand APs over the kc-major xT layout for q range [qoff, qoff+qlen)."""
    view = bsb[:, XT_OFF:XT_OFF + 4 * SP].rearrange("p (kc f c) -> p kc f c", f=4, c=128)
    out = []
    kc0, nfull, rem = qoff // 128, qlen // 128, qlen % 128
    if nfull:
        out.append((0, nfull * 128, view[:, kc0:kc0 + nfull, f, :]))
    if rem:
        out.append((nfull * 128, rem, view[:, kc0 + nfull, f, :rem]))
    return out


def _emit2(nc, SP, SPL, NKC, WV_OFF, WK_OFF, WQ_OFF, XT_OFF, ETE_OFF,
           LIVE2_OFF, COLS, blob, outp, sing, hsb_pool, attp, ps, abl="full"):
    # attention q chunks (512-wide to match one psum bank per head slot)
    QC = []
    off = 0
    while off < SPL:
        ln = min(512, SPL - off)
        QC.append((off, ln))
        off += ln
    NQC = len(QC)

    # projection chunks: k over all SP keys; q over SPL live queries
    KCH = _kchunks(SP)
    SPL_AL = (SPL // 128) * 128
    QPCH = _kchunks(SPL_AL) + ([(SPL_AL, SPL - SPL_AL)] if SPL > SPL_AL else [])

    bsb = sing.tile([128, COLS], MMDT)
    qk = sing.tile([128, 4, SP], MMDT)  # slots: qT cc0, qT cc1, kT cc0, kT cc1
    vall = sing.tile([128, NKC, 65 * HPC], BF)
    scr_d = sing.tile([1, 1], MMDT)

    # ---- input DMAs over both HWDGE rings, ordered so the projection
    # prefix (v, k cc2, q cc0 over the first xT group) unblocks earliest.
    GRP = [(g, min(g + 4, NKC)) for g in range(0, NKC, 4)]  # xT kc groups of 4

    def xt_cols(k0, k1):
        return XT_OFF + k0 * 512, XT_OFF + k1 * 512

    # sync ring: Wv | Wk | Wq+misc | xT tail groups (odd ones)
    nc.sync.dma_start(out=bsb[:, WV_OFF:WV_OFF + 1024], in_=blob[:, WV_OFF:WV_OFF + 1024])
    nc.sync.dma_start(out=bsb[:, WK_OFF:WK_OFF + 1024], in_=blob[:, WK_OFF:WK_OFF + 1024])
    nc.sync.dma_start(out=bsb[:, WQ_OFF:XT_OFF], in_=blob[:, WQ_OFF:XT_OFF])
    # scalar ring: xT g0 | ete cc0 | xT g2 | ete cc1 ; sync takes g1
    c0, c1 = xt_cols(*GRP[0])
    nc.scalar.dma_start(out=bsb[:, c0:c1], in_=blob[:, c0:c1])
    nc.scalar.dma_start(out=bsb[:, ETE_OFF:ETE_OFF + SP], in_=blob[:, ETE_OFF:ETE_OFF + SP])
    if len(GRP) > 2:
        c0, c1 = xt_cols(*GRP[2])
        nc.scalar.dma_start(out=bsb[:, c0:c1], in_=blob[:, c0:c1])
    if len(GRP) > 1:
        c0, c1 = xt_cols(*GRP[1])
        nc.sync.dma_start(out=bsb[:, c0:c1], in_=blob[:, c0:c1])
    for gi in range(3, len(GRP)):
        c0, c1 = xt_cols(*GRP[gi])
        (nc.scalar if gi % 2 == 0 else nc.sync).dma_start(out=bsb[:, c0:c1], in_=blob[:, c0:c1])
    # ete cc1 on the sync ring: the scalar ring must drain before the exp
    # phase ramps so in-flight DMAs never contend with the ACT queue.
    nc.sync.dma_start(out=bsb[:, ETE_OFF + SP:ETE_OFF + 2 * SP],
                      in_=blob[:, ETE_OFF + SP:ETE_OFF + 2 * SP])

    # Engine preambles: observe the DMA lanes via cheap ops so later
    # instructions need at most one fresh semaphore wait each.
    nc.vector.tensor_copy(scr_d, bsb[0:1, 0:1])
    nc.vector.tensor_copy(scr_d, bsb[0:1, XT_OFF:XT_OFF + 1])
    nc.vector.tensor_copy(scr_d, bsb[0:1, ETE_OFF:ETE_OFF + 1])
    nc.scalar.copy(scr_d, bsb[0:1, 0:1])

    # liveness columns of v_aug: 1.0 for live keys, 0.0 for pads — this is
    # the entire key mask (no bias operand needed on the exp).
    for kc in range(NKC):
        va = vall[:, kc, :].rearrange("p (h c) -> p h c", c=65)
        src = bsb[:, LIVE2_OFF + kc * HPC:LIVE2_OFF + (kc + 1) * HPC]
        nc.vector.tensor_copy(va[:, :, 64:65], src.rearrange("p (h c) -> p h c", c=1))

    if abl == "dmas":
        return

    # ---- projection helpers (psum shared with the scores tiles)
    def sp_tile():
        return ps.tile([128, 2, 512], F32, tag="sp", bufs=2, name="sp")

    def v_proj(kc):
        t = sp_tile()
        for f in range(4):
            base = XT_OFF + (kc * 4 + f) * 128
            nc.tensor.matmul(t[:, 0, :256], bsb[:, base:base + 128],
                             bsb[:, WV_OFF + f * 256:WV_OFF + (f + 1) * 256],
                             start=(f == 0), stop=(f == 3))
        va = vall[:, kc, :].rearrange("p (h c) -> p h c", c=65)
        nc.vector.tensor_copy(va[:, :, 0:64],
                              t[:, 0, :256].rearrange("p (h c) -> p h c", c=64))

    def kq_proj(cc, qoff, qlen):
        t = sp_tile()
        nparts = len(_xt_moving(bsb, XT_OFF, SP, 0, qoff, qlen))
        for pi in range(nparts):
            for f in range(4):
                if cc < 2:
                    woff = WQ_OFF + f * 256 + cc * 128
                else:
                    woff = WK_OFF + f * 256 + (cc - 2) * 128
                loff, llen, ap = _xt_moving(bsb, XT_OFF, SP, f, qoff, qlen)[pi]
                nc.tensor.matmul(t[:, 0, loff:loff + llen], bsb[:, woff:woff + 128],
                                 ap, start=(f == 0), stop=(f == 3))
        if cc < 2:  # q: add etype_emb (which includes bq)
            es = bsb[:, ETE_OFF + cc * SP + qoff:ETE_OFF + cc * SP + qoff + qlen]
            nc.vector.tensor_add(qk[:, cc, qoff:qoff + qlen], t[:, 0, :qlen], es)
        else:
            nc.vector.tensor_copy(qk[:, cc, qoff:qoff + qlen], t[:, 0, :qlen])

    if abl == "proj":
        for kc in range(NKC):
            v_proj(kc)
        for cc in (2, 3):
            for qoff, qlen in KCH:
                kq_proj(cc, qoff, qlen)
        for cc in (0, 1):
            for qoff, qlen in QPCH:
                kq_proj(cc, qoff, qlen)
        return

    # ---- attention steps
    steps = [(hp, qc, kc) for hp in (0, 1) for qc in range(NQC) for kc in range(NKC)]
    DEPTH = 2

    # ---- filler schedule: distribute projection work into the attention
    # steps so the (in-order) PE never idles while ACT churns exps.
    # HARD CONSTRAINT: a filler consumed by scores of step j must be emitted
    # at slot <= j - DEPTH - 1 (the scores of step j are emitted during
    # iteration j - DEPTH, before that iteration's fillers); one consumed by
    # PV of step j needs slot <= j - 1. Earlier emission = earlier PE slot,
    # so also keep fillers no earlier than their xT DMA group can land.
    def grp_of(qoff, qlen):
        return ((qoff + qlen - 1) // 128) // 4

    prefix, fillers = [], {}

    def sched(idx, thunk):
        if idx is None or idx < 0:
            prefix.append(thunk)
        else:
            fillers.setdefault(idx, []).append(thunk)

    def first_step_with_kc(kc):  # first step index whose scores touch kc
        return kc  # (hp0, qc0, kc) is at index kc

    def first_step_with_q(hp, qoff):  # first step reading qT[cc=hp] at qoff
        for i, (shp, sqc, skc) in enumerate(steps):
            if shp == hp and QC[sqc][0] <= qoff < QC[sqc][0] + QC[sqc][1]:
                return i
        return len(steps)

    # v(kc): consumed by PV at step idx kc; also gated by DMA group kc//4.
    for kc in range(NKC):
        sched(None if kc < 4 else kc - 1, lambda kc=kc: v_proj(kc))
    # kT chunks: cc=2 feeds hp0 (deadline-tight), cc=3 feeds hp1 (slack).
    for qoff, qlen in KCH:
        g = grp_of(qoff, qlen)
        j2 = first_step_with_kc(qoff // 128)
        ddl2 = j2 - DEPTH - 1
        sched(None if g == 0 else ddl2, lambda o=qoff, l=qlen: kq_proj(2, o, l))
        j3 = NQC * NKC + qoff // 128  # (hp1, qc0, kc) index
        ddl3 = j3 - DEPTH - 1
        sched(min(ddl3, max(3, 2 * g + 8)), lambda o=qoff, l=qlen: kq_proj(3, o, l))
    # qT chunks: cc=0 feeds hp0, cc=1 feeds hp1.
    for qoff, qlen in QPCH:
        g = grp_of(qoff, qlen)
        j0 = first_step_with_q(0, qoff)
        ddl0 = j0 - DEPTH - 1
        sched(None if g == 0 else min(ddl0, max(5, 2 * g + 9)),
              lambda o=qoff, l=qlen: kq_proj(0, o, l))
        j1 = first_step_with_q(1, qoff)
        ddl1 = j1 - DEPTH - 1
        sched(min(ddl1, max(6, 2 * g + 10)) if g or NQC * NKC > 8 else None,
              lambda o=qoff, l=qlen: kq_proj(1, o, l))

    def scores_mm(step, t):
        hp, qc, kc = step
        qoff, qlen = QC[qc]
        for s in range(2):
            cb = s * 64
            lhsT = qk[cb:cb + 64, 2 + hp, kc * 128:(kc + 1) * 128]
            rhs = qk[cb:cb + 64, hp, qoff:qoff + qlen]
            nc.tensor.matmul(t[:, s, :qlen], lhsT, rhs, start=True, stop=True)

    hts = {}
    hpt = {}
    DEPTH = 2
    spq = []
    for th in prefix:
        th()
    for d in range(min(DEPTH, len(steps))):
        t = sp_tile()
        scores_mm(steps[d], t)
        spq.append(t)
    for i, step in enumerate(steps):
        hp, qc, kc = step
        qoff, qlen = QC[qc]
        sp_cur = spq.pop(0)
        if i + DEPTH < len(steps):
            t = sp_tile()
            scores_mm(steps[i + DEPTH], t)
            spq.append(t)
        att = attp.tile([128, 2, 512], BF, tag="att", name="att")
        if abl != "noexp":
            if qlen == 512:  # contiguous across both head slots: one flat AP
                nc.scalar.activation(att[:].rearrange("p a b -> p (a b)"),
                                     sp_cur[:].rearrange("p a b -> p (a b)"),
                                     mybir.ActivationFunctionType.Exp, scale=0.125)
            else:
                nc.scalar.activation(att[:, :, :qlen], sp_cur[:, :, :qlen],
                                     mybir.ActivationFunctionType.Exp, scale=0.125)
        if abl != "nopv":
            for s in range(2):
                h = 2 * hp + s
                if qc == 0 and kc == 0:
                    hts[h] = hsb_pool.tile([65, SP], F32, tag="hts", name="hts")
                if kc == 0:
                    hpt[s] = ps.tile([65, 512], F32, tag="hp", bufs=2, name="hp")
                nc.tensor.matmul(hpt[s][:, :qlen], vall[:, kc, h * 65:(h + 1) * 65],
                                 att[:, s, :qlen], start=(kc == 0), stop=(kc == NKC - 1))
            if kc == NKC - 1:
                for s in range(2):
                    h = 2 * hp + s
                    nc.vector.tensor_copy(hts[h][:, qoff:qoff + qlen], hpt[s][:, :qlen])
                if qc == NQC - 1:  # head pair done: ship hT (+den row) on the
                    # sync ring (idle by now; the ACT queue must stay clear)
                    for s in range(2):
                        h = 2 * hp + s
                        nc.sync.dma_start(out=outp[h, :, :SPL], in_=hts[h][:, :SPL])
        for th in fillers.get(i, []):
            th()
    # any fillers scheduled past the end
    for i in sorted(fillers):
        if i >= len(steps):
            for th in fillers[i]:
                th()


def _prep_core(core, SP, x, etype_emb, mask, Wq, bq, Wk, bk, Wv, bv):
    NKC = SP // 128
    (WV_OFF, WK_OFF, WQ_OFF, XT_OFF, ETE_OFF, BK_OFF, BV_OFF, ONES_OFF,
     LIVE2_OFF, COLS) = _offsets(SP)
    b, hg = core // 2, core % 2
    c0 = hg * CPC
    idx = np.where(mask[b] == 1)[0]
    Su = len(idx)

    blob = np.zeros((128, COLS), np.float32)
    xs = np.zeros((SP, F), np.float32)
    xs[:Su] = x[b][idx]
    xT = xs.T
    xtb = xT.reshape(4, 128, NKC, 128).transpose(1, 2, 0, 3).reshape(128, NKC * 512)
    blob[:, XT_OFF:XT_OFF + 4 * SP] = xtb
    for f in range(4):
        blob[:, WV_OFF + f * 256:WV_OFF + (f + 1) * 256] = Wv[f * 128:(f + 1) * 128, c0:c0 + CPC]
        blob[:, WK_OFF + f * 256:WK_OFF + (f + 1) * 256] = Wk[f * 128:(f + 1) * 128, c0:c0 + CPC]
        blob[:, WQ_OFF + f * 256:WQ_OFF + (f + 1) * 256] = Wq[f * 128:(f + 1) * 128, c0:c0 + CPC]
    et = np.zeros((SP, CPC), np.float32)
    et[:Su] = etype_emb[b][idx][:, c0:c0 + CPC] + bq[c0:c0 + CPC]
    etT = et.T
    blob[:, ETE_OFF:ETE_OFF + SP] = etT[:128]
    blob[:, ETE_OFF + SP:ETE_OFF + 2 * SP] = etT[128:]
    blob[0, BK_OFF:BK_OFF + CPC] = bk[c0:c0 + CPC]
    blob[0, BV_OFF:BV_OFF + CPC] = bv[c0:c0 + CPC]
    blob[0, ONES_OFF:ONES_OFF + SP] = 1.0
    pos = np.arange(128)[:, None] + 128 * np.arange(NKC)[None, :]
    live = (pos < Su).astype(np.float32)          # [128, NKC]
    blob[:, LIVE2_OFF:LIVE2_OFF + NKC * HPC] = np.repeat(live, HPC, axis=1)

    return {"blob": blob}, idx


def kernel(x, etype_emb, mask, Wq, bq, Wk, bk, Wv, bv):
    global LAST_RESULT
    x = np.asarray(x, np.float32)
    etype_emb = np.asarray(etype_emb, np.float32)
    mask = np.asarray(mask)
    Wq, bq = np.asarray(Wq, np.float32), np.asarray(bq, np.float32)
    Wk, bk = np.asarray(Wk, np.float32), np.asarray(bk, np.float32)
    Wv, bv = np.asarray(Wv, np.float32), np.asarray(bv, np.float32)

    counts = [int((mask[b] == 1).sum()) for b in range(B)]
    SPL = max(2, max(counts))
    SPL += SPL % 2  # fp32r matmuls with 128 contraction rows need even N
    SP = max(128, ((SPL + 127) // 128) * 128)

    nc = _build(SP, SPL=SPL)
    in_maps, idxs = [], []
    for core in range(NCORES):
        m, idx = _prep_core(core, SP, x, etype_emb, mask, Wq, bq, Wk, bk, Wv, bv)
        in_maps.append(m)
        idxs.append(idx)

    # The NTFF trace path needs antenv.axon_hooks, which this container does
    # not ship; make sure a stray BASS_TRACE=1 cannot route us into it.
    os.environ.setdefault("BASS_NEVER_TRACE", "1")
    res = run_bass_kernel_spmd(nc, in_maps, list(range(NCORES)))
    LAST_RESULT = res

    out = np.zeros((B, S, E), np.float32)
    for core in range(NCORES):
        b, hg = core // 2, core % 2
        idx = idxs[core]
        if not len(idx):
            continue
        shard = res.results[core]["out"]  # [HPC, 65, SP]: hT rows + denominator
        for h in range(HPC):
            num = shard[h, :64, :len(idx)]
            den = shard[h, 64, :len(idx)]
            bvh = bv[hg * CPC + h * 64:hg * CPC + (h + 1) * 64]
            out[b][idx, hg * CPC + h * 64:hg * CPC + (h + 1) * 64] = (num / den).T + bvh
    return out


# revision 11
# speedup vs baseline: 1.5254x; 1.2508x over previous
"""Masked multi-head attention on 8 TRN2 NeuronCores.

Sharding: core = (batch b, head-group hg). Each core computes the attention
output for one batch element and 4 of the 8 heads (a 256-wide column slice
of E). Rows with mask==0 are dropped host-side before the kernel runs:
masked queries produce all-zero output rows, and masked keys are excluded
via a zeroed "ones" column in the value matrix (their exp(score) is finite
garbage that multiplies a zero v row and a zero denominator weight), so the
kernel only processes the ~half of S that is live (gathered and padded to a
multiple of 128).

Biases never enter the kernel: bq folds into etype_emb host-side, q.bk is
constant per query so it cancels in softmax exactly, and bv satisfies
h = num/den + bv, added host-side.

Per-core on-chip pipeline (fp32r matmuls, exp on ACT, PV in bf16):
  qT/kT = W.T @ xT          (E-cols on partitions, S free)
  v     = xT.T @ Wv         (S on partitions, DH free) + liveness column
  per step (head-pair hp, q-chunk qc, key-chunk kc):
    sT[128k, 2 heads x 512q] = kT.T @ qT   (two matmuls, one psum tile)
    att = exp(sT/8)                        (ONE wide ACT instruction)
    hT[head] += v_aug.T @ att              (accumulates h' and denominator)
  out = hts rows per head DMA'd once; the host transposes and divides.

Projection matmuls are interleaved into the ACT-bound attention phase as PE
filler so the tensor engine stays continuously busy (HAM stays at 2.4GHz).
"""

import os

import numpy as np

import concourse.bacc as bacc
import concourse.tile as tile
from concourse import mybir
from concourse.bass_utils import run_bass_kernel_spmd

BF = mybir.dt.bfloat16
F32 = mybir.dt.float32
MMDT = mybir.dt.float32r  # fp32 storage, full-rate single-pass PE mode

B, S, F, E, H = 4, 2048, 512, 512, 8
DH = 64
NCORES = 8
HPC = 4            # heads per core
CPC = HPC * DH     # output columns per core

LAST_RESULT = None  # BassKernelResults of the most recent run (for test harness)


def _offsets(SP):
    # W stored v|k|q; xT stored kc-major so progressive DMAs unlock
    # projection chunks as they land
    NKC = SP // 128
    WV_OFF = 0
    WK_OFF = 1024
    WQ_OFF = 2048
    BK_OFF = 3072
    BV_OFF = BK_OFF + 256
    ONES_OFF = BK_OFF + 512
    LIVE2_OFF = ONES_OFF + SP      # [128, NKC*HPC] liveness (v_aug den cols)
    XT_OFF = LIVE2_OFF + NKC * HPC
    ETE_OFF = XT_OFF + 4 * SP
    COLS = ETE_OFF + 2 * SP
    return WV_OFF, WK_OFF, WQ_OFF, XT_OFF, ETE_OFF, BK_OFF, BV_OFF, ONES_OFF, LIVE2_OFF, COLS


def _kchunks(total):
    """128-aligned chunks, each <=512 and (when possible) >=256 so fp32r
    matmuls run at full rate."""
    out, off = [], 0
    while total - off > 512 + 255:
        out.append((off, 512))
        off += 512
    rem = total - off
    if rem > 512:
        a = max(256, (rem // 2 // 128) * 128)
        out.append((off, a))
        out.append((off + a, rem - a))
    elif rem:
        out.append((off, rem))
    return out


def _build(SP, loop_reps=None, abl="full", SPL=None, has_bias=True):
    if SPL is None:
        SPL = SP
    NKC = SP // 128
    (WV_OFF, WK_OFF, WQ_OFF, XT_OFF, ETE_OFF, BK_OFF, BV_OFF, ONES_OFF,
     LIVE2_OFF, COLS) = _offsets(SP)

    nc = bacc.Bacc()
    blob = nc.declare_dram_parameter("blob", [128, COLS], MMDT, isOutput=False)
    outp = nc.declare_dram_parameter("out", [HPC, 65, SP], F32, isOutput=True)

    with tile.TileContext(nc) as tc:
        with (
            tc.tile_pool(name="sing", bufs=1) as sing,
            tc.tile_pool(name="hsb", bufs=4) as hsb_pool,
            tc.tile_pool(name="attp", bufs=4) as attp,
            tc.tile_pool(name="ps", bufs=2, space="PSUM") as ps,
        ):
            def _body():
                _emit2(nc, SP, SPL, NKC, WV_OFF, WK_OFF, WQ_OFF, XT_OFF,
                       ETE_OFF, LIVE2_OFF, COLS, blob, outp, sing, hsb_pool,
                       attp, ps, abl)

            if loop_reps is None:
                _body()
            else:
                with tc.For_i(0, loop_reps, 1):
                    _body()
    nc.compile()
    return nc


def _xt_moving(bsb, XT_OFF, SP, f, qoff, qlen):
    """Moving-operand APs over the kc-major xT layout for q range [qoff, qoff+qlen)."""
    view = bsb[:, XT_OFF:XT_OFF + 4 * SP].rearrange("p (kc f c) -> p kc f c", f=4, c=128)
    out = []
    kc0, nfull, rem = qoff // 128, qlen // 128, qlen % 128
    if nfull:
        out.append((0, nfull * 128, view[:, kc0:kc0 + nfull, f, :]))
    if rem:
        out.append((nfull * 128, rem, view[:, kc0 + nfull, f, :rem]))
    return out


def _emit2(nc, SP, SPL, NKC, WV_OFF, WK_OFF, WQ_OFF, XT_OFF, ETE_OFF,
           LIVE2_OFF, COLS, blob, outp, sing, hsb_pool, attp, ps, abl="full"):
    # attention q chunks (512-wide to match one psum bank per head slot)
    QC = []
    off = 0
    while off < SPL:
        ln = min(512, SPL - off)
        QC.append((off, ln))
        off += ln
    NQC = len(QC)

    # projection chunks: k over all SP keys; q over SPL live queries
    KCH = _kchunks(SP)
    SPL_AL = (SPL // 128) * 128
    QPCH = _kchunks(SPL_AL) + ([(SPL_AL, SPL - SPL_AL)] if SPL > SPL_AL else [])

    # bufs=2 so a For_i iteration's DMAs + projections overlap the previous
    # iteration's (ACT-bound) attention phase instead of serializing on the
    # single buffer's last reader.
    bsb = sing.tile([128, COLS], MMDT, bufs=2, name="bsb")
    qk = sing.tile([128, 4, SP], MMDT, bufs=2, name="qk")  # qT cc0/cc1, kT cc0/cc1
    vall = sing.tile([128, NKC, 65 * HPC], BF, bufs=2, name="vall")
    scr_d = sing.tile([1, 1], MMDT, name="scr_d")

    # ---- input DMAs over both HWDGE rings, ordered so the projection
    # prefix (v, k cc2, q cc0 over the first xT group) unblocks earliest.
    GRP = [(g, min(g + 4, NKC)) for g in range(0, NKC, 4)]  # xT kc groups of 4

    def xt_cols(k0, k1):
        return XT_OFF + k0 * 512, XT_OFF + k1 * 512

    # sync ring: Wv | Wk | Wq+misc | xT tail groups (odd ones)
    nc.sync.dma_start(out=bsb[:, WV_OFF:WV_OFF + 1024], in_=blob[:, WV_OFF:WV_OFF + 1024])
    nc.sync.dma_start(out=bsb[:, WK_OFF:WK_OFF + 1024], in_=blob[:, WK_OFF:WK_OFF + 1024])
    nc.sync.dma_start(out=bsb[:, WQ_OFF:XT_OFF], in_=blob[:, WQ_OFF:XT_OFF])
    # scalar ring: xT g0 | ete cc0 | xT g2 | ete cc1 ; sync takes g1
    c0, c1 = xt_cols(*GRP[0])
    nc.scalar.dma_start(out=bsb[:, c0:c1], in_=blob[:, c0:c1])
    nc.scalar.dma_start(out=bsb[:, ETE_OFF:ETE_OFF + SP], in_=blob[:, ETE_OFF:ETE_OFF + SP])
    if len(GRP) > 2:
        c0, c1 = xt_cols(*GRP[2])
        nc.scalar.dma_start(out=bsb[:, c0:c1], in_=blob[:, c0:c1])
    if len(GRP) > 1:
        c0, c1 = xt_cols(*GRP[1])
        nc.sync.dma_start(out=bsb[:, c0:c1], in_=blob[:, c0:c1])
    for gi in range(3, len(GRP)):
        c0, c1 = xt_cols(*GRP[gi])
        (nc.scalar if gi % 2 == 0 else nc.sync).dma_start(out=bsb[:, c0:c1], in_=blob[:, c0:c1])
    # ete cc1 on the sync ring: the scalar ring must drain before the exp
    # phase ramps so in-flight DMAs never contend with the ACT queue.
    nc.sync.dma_start(out=bsb[:, ETE_OFF + SP:ETE_OFF + 2 * SP],
                      in_=blob[:, ETE_OFF + SP:ETE_OFF + 2 * SP])

    # Engine preambles: observe the DMA lanes via cheap ops so later
    # instructions need at most one fresh semaphore wait each.
    nc.vector.tensor_copy(scr_d, bsb[0:1, 0:1])
    nc.vector.tensor_copy(scr_d, bsb[0:1, XT_OFF:XT_OFF + 1])
    nc.vector.tensor_copy(scr_d, bsb[0:1, ETE_OFF:ETE_OFF + 1])
    nc.scalar.copy(scr_d, bsb[0:1, 0:1])

    # liveness columns of v_aug: 1.0 for live keys, 0.0 for pads — this is
    # the entire key mask (no bias operand needed on the exp).
    for kc in range(NKC):
        va = vall[:, kc, :].rearrange("p (h c) -> p h c", c=65)
        src = bsb[:, LIVE2_OFF + kc * HPC:LIVE2_OFF + (kc + 1) * HPC]
        nc.vector.tensor_copy(va[:, :, 64:65], src.rearrange("p (h c) -> p h c", c=1))

    if abl == "dmas":
        return

    # ---- projection helpers. Projections get their OWN psum tag: sharing a
    # tag with the scores tiles collapses the scores double-buffer rotation
    # (every scores tile would land on the previous scores tile's buffer and
    # serialize the whole exp pipeline behind PE).
    def sp_tile():
        return ps.tile([128, 2, 512], F32, tag="sp", bufs=2, name="sp")

    def pj_tile():
        return ps.tile([128, 512], F32, tag="pj", bufs=2, name="pj")

    def v_proj(kc):
        t = pj_tile()
        for f in range(4):
            base = XT_OFF + (kc * 4 + f) * 128
            nc.tensor.matmul(t[:, :256], bsb[:, base:base + 128],
                             bsb[:, WV_OFF + f * 256:WV_OFF + (f + 1) * 256],
                             start=(f == 0), stop=(f == 3))
        va = vall[:, kc, :].rearrange("p (h c) -> p h c", c=65)
        nc.vector.tensor_copy(va[:, :, 0:64],
                              t[:, :256].rearrange("p (h c) -> p h c", c=64))

    def kq_proj(cc, qoff, qlen):
        t = pj_tile()
        nparts = len(_xt_moving(bsb, XT_OFF, SP, 0, qoff, qlen))
        for pi in range(nparts):
            for f in range(4):
                if cc < 2:
                    woff = WQ_OFF + f * 256 + cc * 128
                else:
                    woff = WK_OFF + f * 256 + (cc - 2) * 128
                loff, llen, ap = _xt_moving(bsb, XT_OFF, SP, f, qoff, qlen)[pi]
                nc.tensor.matmul(t[:, loff:loff + llen], bsb[:, woff:woff + 128],
                                 ap, start=(f == 0), stop=(f == 3))
        if cc < 2:  # q: add etype_emb (which includes bq)
            es = bsb[:, ETE_OFF + cc * SP + qoff:ETE_OFF + cc * SP + qoff + qlen]
            nc.vector.tensor_add(qk[:, cc, qoff:qoff + qlen], t[:, :qlen], es)
        else:
            nc.vector.tensor_copy(qk[:, cc, qoff:qoff + qlen], t[:, :qlen])

    if abl == "proj":
        for kc in range(NKC):
            v_proj(kc)
        for cc in (2, 3):
            for qoff, qlen in KCH:
                kq_proj(cc, qoff, qlen)
        for cc in (0, 1):
            for qoff, qlen in QPCH:
                kq_proj(cc, qoff, qlen)
        return

    # ---- attention steps
    steps = [(hp, qc, kc) for hp in (0, 1) for qc in range(NQC) for kc in range(NKC)]
    DEPTH = 2

    # ---- filler schedule: distribute projection work into the attention
    # steps so the (in-order) PE never idles while ACT churns exps.
    # HARD CONSTRAINT: a filler consumed by scores of step j must be emitted
    # at slot <= j - DEPTH - 1 (the scores of step j are emitted during
    # iteration j - DEPTH, before that iteration's fillers); one consumed by
    # PV of step j needs slot <= j - 1. Earlier emission = earlier PE slot,
    # so also keep fillers no earlier than their xT DMA group can land.
    def grp_of(qoff, qlen):
        return ((qoff + qlen - 1) // 128) // 4

    prefix, fillers = [], {}

    def sched(idx, thunk):
        if idx is None or idx < 0:
            prefix.append(thunk)
        else:
            fillers.setdefault(idx, []).append(thunk)

    def first_step_with_kc(kc):  # first step index whose scores touch kc
        return kc  # (hp0, qc0, kc) is at index kc

    def first_step_with_q(hp, qoff):  # first step reading qT[cc=hp] at qoff
        for i, (shp, sqc, skc) in enumerate(steps):
            if shp == hp and QC[sqc][0] <= qoff < QC[sqc][0] + QC[sqc][1]:
                return i
        return len(steps)

    # v(kc): consumed by PV at step idx kc; also gated by DMA group kc//4.
    for kc in range(NKC):
        sched(None if kc < 4 else kc - 1, lambda kc=kc: v_proj(kc))
    # kT chunks: cc=2 feeds hp0 (deadline-tight), cc=3 feeds hp1 (slack).
    for qoff, qlen in KCH:
        g = grp_of(qoff, qlen)
        j2 = first_step_with_kc(qoff // 128)
        ddl2 = j2 - DEPTH - 1
        sched(None if g == 0 else ddl2, lambda o=qoff, l=qlen: kq_proj(2, o, l))
        j3 = NQC * NKC + qoff // 128  # (hp1, qc0, kc) index
        ddl3 = j3 - DEPTH - 1
        sched(min(ddl3, max(3, 2 * g + 8)), lambda o=qoff, l=qlen: kq_proj(3, o, l))
    # qT chunks: cc=0 feeds hp0, cc=1 feeds hp1.
    for qoff, qlen in QPCH:
        g = grp_of(qoff, qlen)
        j0 = first_step_with_q(0, qoff)
        ddl0 = j0 - DEPTH - 1
        sched(None if g == 0 else min(ddl0, max(5, 2 * g + 9)),
              lambda o=qoff, l=qlen: kq_proj(0, o, l))
        j1 = first_step_with_q(1, qoff)
        ddl1 = j1 - DEPTH - 1
        sched(min(ddl1, max(6, 2 * g + 10)) if g or NQC * NKC > 8 else None,
              lambda o=qoff, l=qlen: kq_proj(1, o, l))

    def scores_mm(step, t):
        hp, qc, kc = step
        qoff, qlen = QC[qc]
        for s in range(2):
            cb = s * 64
            lhsT = qk[cb:cb + 64, 2 + hp, kc * 128:(kc + 1) * 128]
            rhs = qk[cb:cb + 64, hp, qoff:qoff + qlen]
            nc.tensor.matmul(t[:, s, :qlen], lhsT, rhs, start=True, stop=True)

    hts = {}
    hpt = {}
    DEPTH = 2
    spq = []
    for th in prefix:
        th()
    for d in range(min(DEPTH, len(steps))):
        t = sp_tile()
        scores_mm(steps[d], t)
        spq.append(t)
    for i, step in enumerate(steps):
        hp, qc, kc = step
        qoff, qlen = QC[qc]
        sp_cur = spq.pop(0)
        if i + DEPTH < len(steps):
            t = sp_tile()
            scores_mm(steps[i + DEPTH], t)
            spq.append(t)
        att = attp.tile([128, 2, 512], BF, tag="att", name="att")
        if abl != "noexp":
            if qlen == 512:  # contiguous across both head slots: one flat AP
                nc.scalar.activation(att[:].rearrange("p a b -> p (a b)"),
                                     sp_cur[:].rearrange("p a b -> p (a b)"),
                                     mybir.ActivationFunctionType.Exp, scale=0.125)
            else:
                nc.scalar.activation(att[:, :, :qlen], sp_cur[:, :, :qlen],
                                     mybir.ActivationFunctionType.Exp, scale=0.125)
        if abl != "nopv":
            for s in range(2):
                h = 2 * hp + s
                if qc == 0 and kc == 0:
                    hts[h] = hsb_pool.tile([65, SP], F32, tag="hts", name="hts")
                if kc == 0:
                    hpt[s] = ps.tile([65, 512], F32, tag="hp", bufs=2, name="hp")
                nc.tensor.matmul(hpt[s][:, :qlen], vall[:, kc, h * 65:(h + 1) * 65],
                                 att[:, s, :qlen], start=(kc == 0), stop=(kc == NKC - 1))
            if kc == NKC - 1:
                for s in range(2):
                    h = 2 * hp + s
                    nc.vector.tensor_copy(hts[h][:, qoff:qoff + qlen], hpt[s][:, :qlen])
                if qc == NQC - 1:  # head pair done: ship hT (+den row) on the
                    # sync ring (idle by now; the ACT queue must stay clear)
                    for s in range(2):
                        h = 2 * hp + s
                        nc.sync.dma_start(out=outp[h, :, :SPL], in_=hts[h][:, :SPL])
        for th in fillers.get(i, []):
            th()
    # any fillers scheduled past the end
    for i in sorted(fillers):
        if i >= len(steps):
            for th in fillers[i]:
                th()


def _prep_core(core, SP, x, etype_emb, mask, Wq, bq, Wk, bk, Wv, bv):
    NKC = SP // 128
    (WV_OFF, WK_OFF, WQ_OFF, XT_OFF, ETE_OFF, BK_OFF, BV_OFF, ONES_OFF,
     LIVE2_OFF, COLS) = _offsets(SP)
    b, hg = core // 2, core % 2
    c0 = hg * CPC
    idx = np.where(mask[b] == 1)[0]
    Su = len(idx)

    blob = np.zeros((128, COLS), np.float32)
    xs = np.zeros((SP, F), np.float32)
    xs[:Su] = x[b][idx]
    xT = xs.T
    xtb = xT.reshape(4, 128, NKC, 128).transpose(1, 2, 0, 3).reshape(128, NKC * 512)
    blob[:, XT_OFF:XT_OFF + 4 * SP] = xtb
    for f in range(4):
        blob[:, WV_OFF + f * 256:WV_OFF + (f + 1) * 256] = Wv[f * 128:(f + 1) * 128, c0:c0 + CPC]
        blob[:, WK_OFF + f * 256:WK_OFF + (f + 1) * 256] = Wk[f * 128:(f + 1) * 128, c0:c0 + CPC]
        blob[:, WQ_OFF + f * 256:WQ_OFF + (f + 1) * 256] = Wq[f * 128:(f + 1) * 128, c0:c0 + CPC]
    et = np.zeros((SP, CPC), np.float32)
    et[:Su] = etype_emb[b][idx][:, c0:c0 + CPC] + bq[c0:c0 + CPC]
    etT = et.T
    blob[:, ETE_OFF:ETE_OFF + SP] = etT[:128]
    blob[:, ETE_OFF + SP:ETE_OFF + 2 * SP] = etT[128:]
    blob[0, BK_OFF:BK_OFF + CPC] = bk[c0:c0 + CPC]
    blob[0, BV_OFF:BV_OFF + CPC] = bv[c0:c0 + CPC]
    blob[0, ONES_OFF:ONES_OFF + SP] = 1.0
    pos = np.arange(128)[:, None] + 128 * np.arange(NKC)[None, :]
    live = (pos < Su).astype(np.float32)          # [128, NKC]
    blob[:, LIVE2_OFF:LIVE2_OFF + NKC * HPC] = np.repeat(live, HPC, axis=1)

    return {"blob": blob}, idx


def kernel(x, etype_emb, mask, Wq, bq, Wk, bk, Wv, bv):
    global LAST_RESULT
    x = np.asarray(x, np.float32)
    etype_emb = np.asarray(etype_emb, np.float32)
    mask = np.asarray(mask)
    Wq, bq = np.asarray(Wq, np.float32), np.asarray(bq, np.float32)
    Wk, bk = np.asarray(Wk, np.float32), np.asarray(bk, np.float32)
    Wv, bv = np.asarray(Wv, np.float32), np.asarray(bv, np.float32)

    counts = [int((mask[b] == 1).sum()) for b in range(B)]
    SPL = max(2, max(counts))
    SPL += SPL % 2  # fp32r matmuls with 128 contraction rows need even N
    SP = max(128, ((SPL + 127) // 128) * 128)

    nc = _build(SP, SPL=SPL)
    in_maps, idxs = [], []
    for core in range(NCORES):
        m, idx = _prep_core(core, SP, x, etype_emb, mask, Wq, bq, Wk, bk, Wv, bv)
        in_maps.append(m)
        idxs.append(idx)

    # The NTFF trace path needs antenv.axon_hooks, which this container does
    # not ship; make sure a stray BASS_TRACE=1 cannot route us into it.
    os.environ.setdefault("BASS_NEVER_TRACE", "1")
    res = run_bass_kernel_spmd(nc, in_maps, list(range(NCORES)))
    LAST_RESULT = res

    out = np.zeros((B, S, E), np.float32)
    for core in range(NCORES):
        b, hg = core // 2, core % 2
        idx = idxs[core]
        if not len(idx):
            continue
        shard = res.results[core]["out"]  # [HPC, 65, SP]: hT rows + denominator
        for h in range(HPC):
            num = shard[h, :64, :len(idx)]
            den = shard[h, 64, :len(idx)]
            bvh = bv[hg * CPC + h * 64:hg * CPC + (h + 1) * 64]
            out[b][idx, hg * CPC + h * 64:hg * CPC + (h + 1) * 64] = (num / den).T + bvh
    return out


# revision 20
# speedup vs baseline: 1.6082x; 1.0543x over previous
"""Masked multi-head attention on 8 TRN2 NeuronCores.

Sharding: core = (batch b, head-group hg). Each core computes the attention
output for one batch element and 4 of the 8 heads (a 256-wide column slice
of E). Rows with mask==0 are dropped host-side before the kernel runs:
masked queries produce all-zero output rows, and masked keys are excluded
via a zeroed "ones" column in the value matrix (their exp(score) is finite
garbage that multiplies a zero v row and a zero denominator weight), so the
kernel only processes the ~half of S that is live (gathered and padded to a
multiple of 128).

Biases never enter the kernel: bq folds into etype_emb host-side, q.bk is
constant per query so it cancels in softmax exactly, and bv satisfies
h = num/den + bv, added host-side.

Per-core on-chip pipeline (fp32r matmuls, exp on ACT, PV in bf16):
  qT/kT = W.T @ xT          (E-cols on partitions, S free)
  v     = xT.T @ Wv         (S on partitions, DH free) + liveness column
  per step (head-pair hp, q-chunk qc, key-chunk kc):
    sT[128k, 2 heads x 512q] = kT.T @ qT   (two matmuls, one psum tile)
    att = exp(sT/8)                        (ONE wide ACT instruction)
    hT[head] += v_aug.T @ att              (accumulates h' and denominator)
  out = hts rows per head DMA'd once; the host transposes and divides.

Projection matmuls are interleaved into the ACT-bound attention phase as PE
filler so the tensor engine stays continuously busy (HAM stays at 2.4GHz).
"""

import os

import numpy as np

import concourse.bacc as bacc
import concourse.tile as tile
from concourse import mybir
from concourse.bass_utils import run_bass_kernel_spmd

BF = mybir.dt.bfloat16
F32 = mybir.dt.float32
MMDT = mybir.dt.float32r  # fp32 storage, full-rate single-pass PE mode

B, S, F, E, H = 4, 2048, 512, 512, 8
DH = 64
NCORES = 8
HPC = 4            # heads per core
CPC = HPC * DH     # output columns per core

LAST_RESULT = None  # BassKernelResults of the most recent run (for test harness)


def _offsets(SP):
    # W stored v|k|q; xT stored kc-major so progressive DMAs unlock
    # projection chunks as they land
    NKC = SP // 128
    WV_OFF = 0
    WK_OFF = 1024
    WQ_OFF = 2048
    BK_OFF = 3072
    BV_OFF = BK_OFF + 256
    ONES_OFF = BK_OFF + 512
    LIVE2_OFF = ONES_OFF + SP      # [128, NKC*HPC] liveness (v_aug den cols)
    XT_OFF = LIVE2_OFF + NKC * HPC
    ETE_OFF = XT_OFF + 4 * SP
    COLS = ETE_OFF + 2 * SP
    return WV_OFF, WK_OFF, WQ_OFF, XT_OFF, ETE_OFF, BK_OFF, BV_OFF, ONES_OFF, LIVE2_OFF, COLS


def _kchunks(total):
    """128-aligned chunks, each <=512 and (when possible) >=256 so fp32r
    matmuls run at full rate."""
    out, off = [], 0
    while total - off > 512 + 255:
        out.append((off, 512))
        off += 512
    rem = total - off
    if rem > 512:
        a = max(256, (rem // 2 // 128) * 128)
        out.append((off, a))
        out.append((off + a, rem - a))
    elif rem:
        out.append((off, rem))
    return out


def _build(SP, loop_reps=None, abl="full", SPL=None, has_bias=True):
    if SPL is None:
        SPL = SP
    NKC = SP // 128
    (WV_OFF, WK_OFF, WQ_OFF, XT_OFF, ETE_OFF, BK_OFF, BV_OFF, ONES_OFF,
     LIVE2_OFF, COLS) = _offsets(SP)

    nc = bacc.Bacc()
    blob = nc.declare_dram_parameter("blob", [128, COLS], MMDT, isOutput=False)
    outp = nc.declare_dram_parameter("out", [HPC, 65, SP], F32, isOutput=True)

    with tile.TileContext(nc) as tc:
        with (
            tc.tile_pool(name="sing", bufs=1) as sing,
            tc.tile_pool(name="hsb", bufs=4) as hsb_pool,
            tc.tile_pool(name="attp", bufs=4) as attp,
            tc.tile_pool(name="ps", bufs=2, space="PSUM") as ps,
        ):
            def _body():
                _emit2(nc, SP, SPL, NKC, WV_OFF, WK_OFF, WQ_OFF, XT_OFF,
                       ETE_OFF, LIVE2_OFF, COLS, blob, outp, sing, hsb_pool,
                       attp, ps, abl)

            # One-time PE warm-up: ~4.5us of back-to-back matmuls on zeros so
            # the HAM clock gate opens (1.2 -> 2.4 GHz) before the first real
            # matmul. Runs under the initial input-DMA wait; the steady-state
            # loop then keeps the PE busy enough to hold K=8/8.
            zt = sing.tile([128, 640], BF, name="zt")
            nc.vector.memset(zt[:], 0.0)
            wps = ps.tile([128, 512], F32, tag="pj", bufs=2, name="pj")
            for j in range(10):
                nc.tensor.matmul(wps[:, :512], zt[:, :128], zt[:, 128:640],
                                 start=(j == 0), stop=(j == 9))

            if loop_reps is None:
                _body()
            else:
                with tc.For_i(0, loop_reps, 1):
                    _body()
    nc.compile()
    return nc


def _xt_moving(bsb, XT_OFF, SP, f, qoff, qlen):
    """Moving-operand APs over the kc-major xT layout for q range [qoff, qoff+qlen)."""
    view = bsb[:, XT_OFF:XT_OFF + 4 * SP].rearrange("p (kc f c) -> p kc f c", f=4, c=128)
    out = []
    kc0, nfull, rem = qoff // 128, qlen // 128, qlen % 128
    if nfull:
        out.append((0, nfull * 128, view[:, kc0:kc0 + nfull, f, :]))
    if rem:
        out.append((nfull * 128, rem, view[:, kc0 + nfull, f, :rem]))
    return out


def _emit2(nc, SP, SPL, NKC, WV_OFF, WK_OFF, WQ_OFF, XT_OFF, ETE_OFF,
           LIVE2_OFF, COLS, blob, outp, sing, hsb_pool, attp, ps, abl="full"):
    # attention q chunks (512-wide to match one psum bank per head slot).
    # A thin third chunk (the "stragglers" past q=1024) is merged into ONE
    # scores tile + two small exps per head pair instead of NKC tiny steps.
    QC = []
    off = 0
    while off < SPL:
        ln = min(512, SPL - off)
        QC.append((off, ln))
        off += ln
    NQC = len(QC)
    MERGE = NQC == 3 and QC[2][1] * NKC <= 512
    NQCM = 2 if MERGE else NQC

    # projection chunks: k group-aligned with the xT DMA groups (so the kc
    # execution order below never waits on a not-yet-landed group); q over
    # the SPL live queries.
    KCH = [(g * 512, min(512, SP - g * 512)) for g in range((SP + 511) // 512)]
    SPL_AL = (SPL // 128) * 128
    QPCH = _kchunks(SPL_AL) + ([(SPL_AL, SPL - SPL_AL)] if SPL > SPL_AL else [])

    # kc execution order: group 0 first, then the groups in DMA-landing
    # order (g2 arrives on the scalar ring before g1 lands on sync).
    sigma = list(range(min(4, NKC)))
    if NKC > 8:
        sigma += list(range(8, NKC))
    sigma += list(range(4, min(8, NKC)))
    assert sorted(sigma) == list(range(NKC))

    # bufs=2 so a For_i iteration's DMAs + projections overlap the previous
    # iteration's (ACT-bound) attention phase instead of serializing on the
    # single buffer's last reader.
    bsb = sing.tile([128, COLS], MMDT, bufs=2, name="bsb")
    qk = sing.tile([128, 4, SP], MMDT, bufs=2, name="qk")  # qT cc0/cc1, kT cc0/cc1
    vall = sing.tile([128, NKC, 65 * HPC], BF, bufs=2, name="vall")
    scr_d = sing.tile([1, 1], MMDT, name="scr_d")

    # ---- input DMAs over both HWDGE rings, ordered so the projection
    # prefix (v, k cc2, q cc0 over the first xT group) unblocks earliest.
    GRP = [(g, min(g + 4, NKC)) for g in range(0, NKC, 4)]  # xT kc groups of 4

    def xt_cols(k0, k1):
        return XT_OFF + k0 * 512, XT_OFF + k1 * 512

    # sync ring: Wv | Wk | Wq+misc | xT tail groups (odd ones)
    nc.sync.dma_start(out=bsb[:, WV_OFF:WV_OFF + 1024], in_=blob[:, WV_OFF:WV_OFF + 1024])
    nc.sync.dma_start(out=bsb[:, WK_OFF:WK_OFF + 1024], in_=blob[:, WK_OFF:WK_OFF + 1024])
    nc.sync.dma_start(out=bsb[:, WQ_OFF:XT_OFF], in_=blob[:, WQ_OFF:XT_OFF])
    # scalar ring: xT g0 | ete cc0 | xT g2 | ete cc1 ; sync takes g1
    c0, c1 = xt_cols(*GRP[0])
    nc.scalar.dma_start(out=bsb[:, c0:c1], in_=blob[:, c0:c1])
    nc.scalar.dma_start(out=bsb[:, ETE_OFF:ETE_OFF + SP], in_=blob[:, ETE_OFF:ETE_OFF + SP])
    if len(GRP) > 2:
        c0, c1 = xt_cols(*GRP[2])
        nc.scalar.dma_start(out=bsb[:, c0:c1], in_=blob[:, c0:c1])
    if len(GRP) > 1:
        c0, c1 = xt_cols(*GRP[1])
        nc.sync.dma_start(out=bsb[:, c0:c1], in_=blob[:, c0:c1])
    for gi in range(3, len(GRP)):
        c0, c1 = xt_cols(*GRP[gi])
        (nc.scalar if gi % 2 == 0 else nc.sync).dma_start(out=bsb[:, c0:c1], in_=blob[:, c0:c1])
    # ete cc1 on the sync ring: the scalar ring must drain before the exp
    # phase ramps so in-flight DMAs never contend with the ACT queue.
    nc.sync.dma_start(out=bsb[:, ETE_OFF + SP:ETE_OFF + 2 * SP],
                      in_=blob[:, ETE_OFF + SP:ETE_OFF + 2 * SP])

    # Engine preambles: observe the DMA lanes via cheap ops so later
    # instructions need at most one fresh semaphore wait each.
    nc.vector.tensor_copy(scr_d, bsb[0:1, 0:1])
    nc.vector.tensor_copy(scr_d, bsb[0:1, XT_OFF:XT_OFF + 1])
    nc.vector.tensor_copy(scr_d, bsb[0:1, ETE_OFF:ETE_OFF + 1])
    nc.scalar.copy(scr_d, bsb[0:1, 0:1])

    # liveness columns of v_aug: 1.0 for live keys, 0.0 for pads — this is
    # the entire key mask (no bias operand needed on the exp).
    for kc in range(NKC):
        va = vall[:, kc, :].rearrange("p (h c) -> p h c", c=65)
        src = bsb[:, LIVE2_OFF + kc * HPC:LIVE2_OFF + (kc + 1) * HPC]
        nc.vector.tensor_copy(va[:, :, 64:65], src.rearrange("p (h c) -> p h c", c=1))

    if abl == "dmas":
        return

    if abl == "acts":  # pure ACT throughput probe: the exp chain alone,
        # reading bsb with scale=0 (exp(0)=1, numerics-safe)
        for hp in (0, 1):
            for qc in range(NQC):
                qlen = QC[qc][1]
                W = 512 + qlen if qlen == 512 else 2 * qlen
                for kc in range(NKC):
                    att = attp.tile([128, 2, 512], BF, tag="att", name="att")
                    nc.scalar.activation(att[:].rearrange("p a b -> p (a b)")[:, :W],
                                         bsb[:, :W],
                                         mybir.ActivationFunctionType.Exp, scale=0.0)
        return

    # ---- projection helpers. Projections get their OWN psum tag: sharing a
    # tag with the scores tiles collapses the scores double-buffer rotation
    # (every scores tile would land on the previous scores tile's buffer and
    # serialize the whole exp pipeline behind PE).
    def sp_tile():
        return ps.tile([128, 2, 512], F32, tag="sp", bufs=2, name="sp")

    def pj_tile():
        return ps.tile([128, 512], F32, tag="pj", bufs=2, name="pj")

    def v_proj(kc):
        t = pj_tile()
        for f in range(4):
            base = XT_OFF + (kc * 4 + f) * 128
            nc.tensor.matmul(t[:, :256], bsb[:, base:base + 128],
                             bsb[:, WV_OFF + f * 256:WV_OFF + (f + 1) * 256],
                             start=(f == 0), stop=(f == 3))
        va = vall[:, kc, :].rearrange("p (h c) -> p h c", c=65)
        nc.vector.tensor_copy(va[:, :, 0:64],
                              t[:, :256].rearrange("p (h c) -> p h c", c=64))

    def kq_proj(cc, qoff, qlen):
        t = pj_tile()
        nparts = len(_xt_moving(bsb, XT_OFF, SP, 0, qoff, qlen))
        for pi in range(nparts):
            for f in range(4):
                if cc < 2:
                    woff = WQ_OFF + f * 256 + cc * 128
                else:
                    woff = WK_OFF + f * 256 + (cc - 2) * 128
                loff, llen, ap = _xt_moving(bsb, XT_OFF, SP, f, qoff, qlen)[pi]
                nc.tensor.matmul(t[:, loff:loff + llen], bsb[:, woff:woff + 128],
                                 ap, start=(f == 0), stop=(f == 3))
        if cc < 2:  # q: add etype_emb (which includes bq)
            es = bsb[:, ETE_OFF + cc * SP + qoff:ETE_OFF + cc * SP + qoff + qlen]
            nc.vector.tensor_add(qk[:, cc, qoff:qoff + qlen], t[:, :qlen], es)
        else:
            nc.vector.tensor_copy(qk[:, cc, qoff:qoff + qlen], t[:, :qlen])

    if abl == "proj":
        for kc in range(NKC):
            v_proj(kc)
        for cc in (2, 3):
            for qoff, qlen in KCH:
                kq_proj(cc, qoff, qlen)
        for cc in (0, 1):
            for qoff, qlen in QPCH:
                kq_proj(cc, qoff, qlen)
        return

    # ---- attention steps; kc == -1 marks a head-pair's merged straggler step
    steps = []
    for hp in (0, 1):
        for qc in range(NQCM):
            steps += [(hp, qc, kc) for kc in sigma]
        if MERGE:
            steps.append((hp, 2, -1))
    DEPTH = 2

    # ---- filler schedule: distribute projection work into the attention
    # steps so the (in-order) PE never idles while ACT churns exps.
    # HARD CONSTRAINT: a filler consumed by scores of step j must be emitted
    # at slot <= j - DEPTH - 1 (the scores of step j are emitted during
    # iteration j - DEPTH, before that iteration's fillers); one consumed by
    # PV of step j needs slot <= j - 1. Earlier emission = earlier PE slot,
    # so also keep fillers no earlier than their xT DMA group can land.
    def grp_of(qoff, qlen):
        return ((qoff + qlen - 1) // 128) // 4

    prefix, fillers = [], {}

    def sched(idx, thunk):
        if idx is None or idx < 0 or abl == "nofill":
            prefix.append(thunk)
        else:
            fillers.setdefault(idx, []).append(thunk)

    STEPS_PER_HP = len(steps) // 2

    def first_step_with_kc(kc):  # first step index whose scores touch kc
        return sigma.index(kc)

    def first_step_with_q(hp, qoff):  # first step reading qT[cc=hp] at qoff
        for i, (shp, sqc, skc) in enumerate(steps):
            if shp == hp and QC[sqc][0] <= qoff < QC[sqc][0] + QC[sqc][1]:
                return i
        return len(steps)

    # v(kc): consumed by PV at step idx sigma.index(kc).
    for kc in range(NKC):
        j = first_step_with_kc(kc)
        sched(None if j < 4 else j - 1, lambda kc=kc: v_proj(kc))
    # kT chunks: cc=2 feeds hp0 (deadline-tight), cc=3 feeds hp1 (slack).
    for qoff, qlen in KCH:
        g = grp_of(qoff, qlen)
        j2 = min(first_step_with_kc(kc) for kc in range(qoff // 128, (qoff + qlen) // 128))
        ddl2 = j2 - DEPTH - 1
        sched(None if g == 0 else ddl2, lambda o=qoff, l=qlen: kq_proj(2, o, l))
        ddl3 = STEPS_PER_HP + j2 - DEPTH - 1
        sched(min(ddl3, max(3, 2 * g + 8)), lambda o=qoff, l=qlen: kq_proj(3, o, l))
    # qT chunks: cc=0 feeds hp0, cc=1 feeds hp1.
    for qoff, qlen in QPCH:
        g = grp_of(qoff, qlen)
        ddl0 = first_step_with_q(0, qoff) - DEPTH - 1
        sched(None if g == 0 else min(ddl0, max(5, 2 * g + 9)),
              lambda o=qoff, l=qlen: kq_proj(0, o, l))
        ddl1 = first_step_with_q(1, qoff) - DEPTH - 1
        sched(min(ddl1, max(6, 2 * g + 10)) if g or STEPS_PER_HP > 8 else None,
              lambda o=qoff, l=qlen: kq_proj(1, o, l))

    def scores_mm(step, t):
        hp, qc, kc = step
        if kc < 0:  # merged straggler step: all NKC key chunks, thin q tail
            qoff, SWl = QC[2]
            for s in range(2):
                cb = s * 64
                rhs = qk[cb:cb + 64, hp, qoff:qoff + SWl]
                for k2 in range(NKC):
                    lhsT = qk[cb:cb + 64, 2 + hp, k2 * 128:(k2 + 1) * 128]
                    nc.tensor.matmul(t[:, s, k2 * SWl:(k2 + 1) * SWl], lhsT, rhs,
                                     start=True, stop=True)
            return
        qoff, qlen = QC[qc]
        for s in range(2):
            cb = s * 64
            lhsT = qk[cb:cb + 64, 2 + hp, kc * 128:(kc + 1) * 128]
            rhs = qk[cb:cb + 64, hp, qoff:qoff + qlen]
            nc.tensor.matmul(t[:, s, :qlen], lhsT, rhs, start=True, stop=True)

    hts = {}
    hpt = {}
    DEPTH = 2
    spq = []
    for th in prefix:
        th()
    for d in range(min(DEPTH, len(steps))):
        t = sp_tile()
        scores_mm(steps[d], t)
        spq.append(t)
    last_kc = sigma[-1]
    for i, step in enumerate(steps):
        hp, qc, kc = step
        qoff, qlen = QC[qc]
        sp_cur = spq.pop(0)
        if i + DEPTH < len(steps):
            t = sp_tile()
            scores_mm(steps[i + DEPTH], t)
            spq.append(t)
        att = attp.tile([128, 2, 512], BF, tag="att", name="att")
        if kc < 0:  # merged straggler step: one exp + PV chain per head
            SWl = qlen
            for s in range(2):
                h = 2 * hp + s
                if abl != "noexp":
                    nc.scalar.activation(att[:, s, :NKC * SWl], sp_cur[:, s, :NKC * SWl],
                                         mybir.ActivationFunctionType.Exp, scale=0.125)
                if abl == "nopv":
                    continue
                hpt[s] = ps.tile([65, 512], F32, tag="hp", bufs=2, name="hp")
                for k2 in range(NKC):
                    nc.tensor.matmul(hpt[s][:, :SWl], vall[:, k2, h * 65:(h + 1) * 65],
                                     att[:, s, k2 * SWl:(k2 + 1) * SWl],
                                     start=(k2 == 0), stop=(k2 == NKC - 1))
                nc.vector.tensor_copy(hts[h][:, qoff:qoff + SWl], hpt[s][:, :SWl])
                nc.sync.dma_start(out=outp[h, :, :SPL], in_=hts[h][:, :SPL])
            for th in fillers.get(i, []):
                th()
            continue
        if abl != "noexp":
            if qlen == 512:  # contiguous across both head slots: one flat AP
                nc.scalar.activation(att[:].rearrange("p a b -> p (a b)"),
                                     sp_cur[:].rearrange("p a b -> p (a b)"),
                                     mybir.ActivationFunctionType.Exp, scale=0.125)
            else:
                nc.scalar.activation(att[:, :, :qlen], sp_cur[:, :, :qlen],
                                     mybir.ActivationFunctionType.Exp, scale=0.125)
        if abl != "nopv":
            for s in range(2):
                h = 2 * hp + s
                if qc == 0 and kc == 0:
                    hts[h] = hsb_pool.tile([65, SP], F32, tag="hts", name="hts")
                if kc == 0:
                    hpt[s] = ps.tile([65, 512], F32, tag="hp", bufs=2, name="hp")
                nc.tensor.matmul(hpt[s][:, :qlen], vall[:, kc, h * 65:(h + 1) * 65],
                                 att[:, s, :qlen], start=(kc == 0), stop=(kc == last_kc))
            if kc == last_kc:
                for s in range(2):
                    h = 2 * hp + s
                    nc.vector.tensor_copy(hts[h][:, qoff:qoff + qlen], hpt[s][:, :qlen])
                if qc == NQCM - 1 and not MERGE:  # ship hT (+den row) on the
                    # sync ring (idle by now; the ACT queue must stay clear)
                    for s in range(2):
                        h = 2 * hp + s
                        nc.sync.dma_start(out=outp[h, :, :SPL], in_=hts[h][:, :SPL])
        for th in fillers.get(i, []):
            th()
    # any fillers scheduled past the end
    for i in sorted(fillers):
        if i >= len(steps):
            for th in fillers[i]:
                th()


def _prep_core(core, SP, x, etype_emb, mask, Wq, bq, Wk, bk, Wv, bv):
    NKC = SP // 128
    (WV_OFF, WK_OFF, WQ_OFF, XT_OFF, ETE_OFF, BK_OFF, BV_OFF, ONES_OFF,
     LIVE2_OFF, COLS) = _offsets(SP)
    b, hg = core // 2, core % 2
    c0 = hg * CPC
    idx = np.where(mask[b] == 1)[0]
    Su = len(idx)

    blob = np.zeros((128, COLS), np.float32)
    xs = np.zeros((SP, F), np.float32)
    xs[:Su] = x[b][idx]
    xT = xs.T
    xtb = xT.reshape(4, 128, NKC, 128).transpose(1, 2, 0, 3).reshape(128, NKC * 512)
    blob[:, XT_OFF:XT_OFF + 4 * SP] = xtb
    for f in range(4):
        blob[:, WV_OFF + f * 256:WV_OFF + (f + 1) * 256] = Wv[f * 128:(f + 1) * 128, c0:c0 + CPC]
        blob[:, WK_OFF + f * 256:WK_OFF + (f + 1) * 256] = Wk[f * 128:(f + 1) * 128, c0:c0 + CPC]
        blob[:, WQ_OFF + f * 256:WQ_OFF + (f + 1) * 256] = Wq[f * 128:(f + 1) * 128, c0:c0 + CPC]
    et = np.zeros((SP, CPC), np.float32)
    et[:Su] = etype_emb[b][idx][:, c0:c0 + CPC] + bq[c0:c0 + CPC]
    etT = et.T
    blob[:, ETE_OFF:ETE_OFF + SP] = etT[:128]
    blob[:, ETE_OFF + SP:ETE_OFF + 2 * SP] = etT[128:]
    blob[0, BK_OFF:BK_OFF + CPC] = bk[c0:c0 + CPC]
    blob[0, BV_OFF:BV_OFF + CPC] = bv[c0:c0 + CPC]
    blob[0, ONES_OFF:ONES_OFF + SP] = 1.0
    pos = np.arange(128)[:, None] + 128 * np.arange(NKC)[None, :]
    live = (pos < Su).astype(np.float32)          # [128, NKC]
    blob[:, LIVE2_OFF:LIVE2_OFF + NKC * HPC] = np.repeat(live, HPC, axis=1)

    return {"blob": blob}, idx


def kernel(x, etype_emb, mask, Wq, bq, Wk, bk, Wv, bv):
    global LAST_RESULT
    x = np.asarray(x, np.float32)
    etype_emb = np.asarray(etype_emb, np.float32)
    mask = np.asarray(mask)
    Wq, bq = np.asarray(Wq, np.float32), np.asarray(bq, np.float32)
    Wk, bk = np.asarray(Wk, np.float32), np.asarray(bk, np.float32)
    Wv, bv = np.asarray(Wv, np.float32), np.asarray(bv, np.float32)

    counts = [int((mask[b] == 1).sum()) for b in range(B)]
    SPL = max(2, max(counts))
    SPL += SPL % 2  # fp32r matmuls with 128 contraction rows need even N
    SP = max(128, ((SPL + 127) // 128) * 128)

    nc = _build(SP, SPL=SPL)
    in_maps, idxs = [], []
    for core in range(NCORES):
        m, idx = _prep_core(core, SP, x, etype_emb, mask, Wq, bq, Wk, bk, Wv, bv)
        in_maps.append(m)
        idxs.append(idx)

    # The NTFF trace path needs antenv.axon_hooks, which this container does
    # not ship; make sure a stray BASS_TRACE=1 cannot route us into it.
    os.environ.setdefault("BASS_NEVER_TRACE", "1")
    res = run_bass_kernel_spmd(nc, in_maps, list(range(NCORES)))
    LAST_RESULT = res

    out = np.zeros((B, S, E), np.float32)
    for core in range(NCORES):
        b, hg = core // 2, core % 2
        idx = idxs[core]
        if not len(idx):
            continue
        shard = res.results[core]["out"]  # [HPC, 65, SP]: hT rows + denominator
        for h in range(HPC):
            num = shard[h, :64, :len(idx)]
            den = shard[h, 64, :len(idx)]
            bvh = bv[hg * CPC + h * 64:hg * CPC + (h + 1) * 64]
            out[b][idx, hg * CPC + h * 64:hg * CPC + (h + 1) * 64] = (num / den).T + bvh
    return out


# revision 22
# speedup vs baseline: 1.6278x; 1.0122x over previous
"""Masked multi-head attention on 8 TRN2 NeuronCores.

Sharding: core = (batch b, head-group hg). Each core computes the attention
output for one batch element and 4 of the 8 heads (a 256-wide column slice
of E). Rows with mask==0 are dropped host-side before the kernel runs:
masked queries produce all-zero output rows, and masked keys are excluded
via a zeroed "ones" column in the value matrix (their exp(score) is finite
garbage that multiplies a zero v row and a zero denominator weight), so the
kernel only processes the ~half of S that is live (gathered and padded to a
multiple of 128).

Biases never enter the kernel: bq folds into etype_emb host-side, q.bk is
constant per query so it cancels in softmax exactly, and bv satisfies
h = num/den + bv, added host-side.

Per-core on-chip pipeline (fp32r matmuls, exp on ACT, PV in bf16):
  qT/kT = W.T @ xT          (E-cols on partitions, S free)
  v     = xT.T @ Wv         (S on partitions, DH free) + liveness column
  per step (head-pair hp, q-chunk qc, key-chunk kc):
    sT[128k, 2 heads x 512q] = kT.T @ qT   (two matmuls, one psum tile)
    att = exp(sT/8)                        (ONE wide ACT instruction)
    hT[head] += v_aug.T @ att              (accumulates h' and denominator)
  out = hts rows per head DMA'd once; the host transposes and divides.

Projection matmuls are interleaved into the ACT-bound attention phase as PE
filler so the tensor engine stays continuously busy (HAM stays at 2.4GHz).
"""

import os

import numpy as np

import concourse.bacc as bacc
import concourse.tile as tile
from concourse import mybir
from concourse.bass_utils import run_bass_kernel_spmd

BF = mybir.dt.bfloat16
F32 = mybir.dt.float32
MMDT = mybir.dt.float32r  # fp32 storage, full-rate single-pass PE mode

B, S, F, E, H = 4, 2048, 512, 512, 8
DH = 64
NCORES = 8
HPC = 4            # heads per core
CPC = HPC * DH     # output columns per core

LAST_RESULT = None  # BassKernelResults of the most recent run (for test harness)


def _offsets(SP):
    # W stored v|k|q; xT stored kc-major so progressive DMAs unlock
    # projection chunks as they land
    NKC = SP // 128
    WV_OFF = 0
    WK_OFF = 1024
    WQ_OFF = 2048
    BK_OFF = 3072
    BV_OFF = BK_OFF + 256
    ONES_OFF = BK_OFF + 512
    LIVE2_OFF = ONES_OFF + SP      # [128, NKC*HPC] liveness (v_aug den cols)
    XT_OFF = LIVE2_OFF + NKC * HPC
    ETE_OFF = XT_OFF + 4 * SP
    COLS = ETE_OFF + 2 * SP
    return WV_OFF, WK_OFF, WQ_OFF, XT_OFF, ETE_OFF, BK_OFF, BV_OFF, ONES_OFF, LIVE2_OFF, COLS


def _kchunks(total):
    """128-aligned chunks, each <=512 and (when possible) >=256 so fp32r
    matmuls run at full rate."""
    out, off = [], 0
    while total - off > 512 + 255:
        out.append((off, 512))
        off += 512
    rem = total - off
    if rem > 512:
        a = max(256, (rem // 2 // 128) * 128)
        out.append((off, a))
        out.append((off + a, rem - a))
    elif rem:
        out.append((off, rem))
    return out


def _build(SP, loop_reps=None, abl="full", SPL=None, has_bias=True, pad=None):
    if pad is None:
        pad = int(os.environ.get("PAD", "0"))
    if SPL is None:
        SPL = SP
    NKC = SP // 128
    (WV_OFF, WK_OFF, WQ_OFF, XT_OFF, ETE_OFF, BK_OFF, BV_OFF, ONES_OFF,
     LIVE2_OFF, COLS) = _offsets(SP)

    nc = bacc.Bacc()
    blob = nc.declare_dram_parameter("blob", [128, COLS], MMDT, isOutput=False)
    outp = nc.declare_dram_parameter("out", [HPC, 65, SP], F32, isOutput=True)

    with tile.TileContext(nc) as tc:
        with (
            tc.tile_pool(name="sing", bufs=1) as sing,
            tc.tile_pool(name="hsb", bufs=4) as hsb_pool,
            tc.tile_pool(name="attp", bufs=4) as attp,
            tc.tile_pool(name="ps", bufs=2, space="PSUM") as ps,
        ):
            zt = sing.tile([128, 640], BF, name="zt")

            def _body():
                _emit2(nc, SP, SPL, NKC, WV_OFF, WK_OFF, WQ_OFF, XT_OFF,
                       ETE_OFF, LIVE2_OFF, COLS, blob, outp, sing, hsb_pool,
                       attp, ps, abl, pad, zt)

            # One-time PE warm-up: ~4.5us of back-to-back matmuls on zeros so
            # the HAM clock gate opens (1.2 -> 2.4 GHz) before the first real
            # matmul. Runs under the initial input-DMA wait; the steady-state
            # loop then keeps the PE busy enough to hold K=8/8.
            nc.vector.memset(zt[:], 0.0)
            wps = ps.tile([128, 512], F32, tag="pj", bufs=2, name="pj")
            for j in range(10):
                nc.tensor.matmul(wps[:, :512], zt[:, :128], zt[:, 128:640],
                                 start=(j == 0), stop=(j == 9))

            if loop_reps is None:
                _body()
            else:
                with tc.For_i(0, loop_reps, 1):
                    _body()
    nc.compile()
    return nc


def _xt_moving(bsb, XT_OFF, SP, f, qoff, qlen):
    """Moving-operand APs over the kc-major xT layout for q range [qoff, qoff+qlen)."""
    view = bsb[:, XT_OFF:XT_OFF + 4 * SP].rearrange("p (kc f c) -> p kc f c", f=4, c=128)
    out = []
    kc0, nfull, rem = qoff // 128, qlen // 128, qlen % 128
    if nfull:
        out.append((0, nfull * 128, view[:, kc0:kc0 + nfull, f, :]))
    if rem:
        out.append((nfull * 128, rem, view[:, kc0 + nfull, f, :rem]))
    return out


def _emit2(nc, SP, SPL, NKC, WV_OFF, WK_OFF, WQ_OFF, XT_OFF, ETE_OFF,
           LIVE2_OFF, COLS, blob, outp, sing, hsb_pool, attp, ps, abl="full",
           pad=0, zt=None):
    # attention q chunks (512-wide to match one psum bank per head slot).
    # A thin third chunk (the "stragglers" past q=1024) is merged into ONE
    # scores tile + two small exps per head pair instead of NKC tiny steps.
    QC = []
    off = 0
    while off < SPL:
        ln = min(512, SPL - off)
        QC.append((off, ln))
        off += ln
    NQC = len(QC)
    MERGE = NQC == 3 and QC[2][1] * NKC <= 512
    NQCM = 2 if MERGE else NQC

    # projection chunks: k group-aligned with the xT DMA groups (so the kc
    # execution order below never waits on a not-yet-landed group); q over
    # the SPL live queries.
    KCH = [(g * 512, min(512, SP - g * 512)) for g in range((SP + 511) // 512)]
    SPL_AL = (SPL // 128) * 128
    QPCH = _kchunks(SPL_AL) + ([(SPL_AL, SPL - SPL_AL)] if SPL > SPL_AL else [])

    # kc execution order: group 0 first, then the groups in DMA-landing
    # order (g2 arrives on the scalar ring before g1 lands on sync).
    sigma = list(range(min(4, NKC)))
    if NKC > 8:
        sigma += list(range(8, NKC))
    sigma += list(range(4, min(8, NKC)))
    assert sorted(sigma) == list(range(NKC))

    # bufs=2 so a For_i iteration's DMAs + projections overlap the previous
    # iteration's (ACT-bound) attention phase instead of serializing on the
    # single buffer's last reader.
    bsb = sing.tile([128, COLS], MMDT, bufs=2, name="bsb")
    qk = sing.tile([128, 4, SP], MMDT, bufs=2, name="qk")  # qT cc0/cc1, kT cc0/cc1
    vall = sing.tile([128, NKC, 65 * HPC], BF, bufs=2, name="vall")
    scr_d = sing.tile([1, 1], MMDT, name="scr_d")

    # ---- input DMAs over both HWDGE rings, ordered so the projection
    # prefix (v, k cc2, q cc0 over the first xT group) unblocks earliest.
    GRP = [(g, min(g + 4, NKC)) for g in range(0, NKC, 4)]  # xT kc groups of 4

    def xt_cols(k0, k1):
        return XT_OFF + k0 * 512, XT_OFF + k1 * 512

    # sync ring: Wv | Wk | Wq+misc | xT tail groups (odd ones)
    nc.sync.dma_start(out=bsb[:, WV_OFF:WV_OFF + 1024], in_=blob[:, WV_OFF:WV_OFF + 1024])
    nc.sync.dma_start(out=bsb[:, WK_OFF:WK_OFF + 1024], in_=blob[:, WK_OFF:WK_OFF + 1024])
    nc.sync.dma_start(out=bsb[:, WQ_OFF:XT_OFF], in_=blob[:, WQ_OFF:XT_OFF])
    # scalar ring: xT g0 | ete cc0 | xT g2 | ete cc1 ; sync takes g1
    c0, c1 = xt_cols(*GRP[0])
    nc.scalar.dma_start(out=bsb[:, c0:c1], in_=blob[:, c0:c1])
    nc.scalar.dma_start(out=bsb[:, ETE_OFF:ETE_OFF + SP], in_=blob[:, ETE_OFF:ETE_OFF + SP])
    if len(GRP) > 2:
        c0, c1 = xt_cols(*GRP[2])
        nc.scalar.dma_start(out=bsb[:, c0:c1], in_=blob[:, c0:c1])
    if len(GRP) > 1:
        c0, c1 = xt_cols(*GRP[1])
        nc.sync.dma_start(out=bsb[:, c0:c1], in_=blob[:, c0:c1])
    for gi in range(3, len(GRP)):
        c0, c1 = xt_cols(*GRP[gi])
        (nc.scalar if gi % 2 == 0 else nc.sync).dma_start(out=bsb[:, c0:c1], in_=blob[:, c0:c1])
    # ete cc1 on the sync ring: the scalar ring must drain before the exp
    # phase ramps so in-flight DMAs never contend with the ACT queue.
    nc.sync.dma_start(out=bsb[:, ETE_OFF + SP:ETE_OFF + 2 * SP],
                      in_=blob[:, ETE_OFF + SP:ETE_OFF + 2 * SP])

    # Engine preambles: observe the DMA lanes via cheap ops so later
    # instructions need at most one fresh semaphore wait each.
    nc.vector.tensor_copy(scr_d, bsb[0:1, 0:1])
    nc.vector.tensor_copy(scr_d, bsb[0:1, XT_OFF:XT_OFF + 1])
    nc.vector.tensor_copy(scr_d, bsb[0:1, ETE_OFF:ETE_OFF + 1])
    nc.scalar.copy(scr_d, bsb[0:1, 0:1])

    # liveness columns of v_aug: 1.0 for live keys, 0.0 for pads — this is
    # the entire key mask (no bias operand needed on the exp).
    for kc in range(NKC):
        va = vall[:, kc, :].rearrange("p (h c) -> p h c", c=65)
        src = bsb[:, LIVE2_OFF + kc * HPC:LIVE2_OFF + (kc + 1) * HPC]
        nc.vector.tensor_copy(va[:, :, 64:65], src.rearrange("p (h c) -> p h c", c=1))

    if abl == "dmas":
        return

    if abl == "acts":  # pure ACT throughput probe: the exp chain alone,
        # reading bsb with scale=0 (exp(0)=1, numerics-safe)
        for hp in (0, 1):
            for qc in range(NQC):
                qlen = QC[qc][1]
                W = 512 + qlen if qlen == 512 else 2 * qlen
                for kc in range(NKC):
                    att = attp.tile([128, 2, 512], BF, tag="att", name="att")
                    nc.scalar.activation(att[:].rearrange("p a b -> p (a b)")[:, :W],
                                         bsb[:, :W],
                                         mybir.ActivationFunctionType.Exp, scale=0.0)
        return

    # ---- projection helpers. Projections get their OWN psum tag: sharing a
    # tag with the scores tiles collapses the scores double-buffer rotation
    # (every scores tile would land on the previous scores tile's buffer and
    # serialize the whole exp pipeline behind PE).
    def sp_tile():
        return ps.tile([128, 2, 512], F32, tag="sp", bufs=2, name="sp")

    def pj_tile():
        return ps.tile([128, 512], F32, tag="pj", bufs=2, name="pj")

    def v_proj(kc):
        t = pj_tile()
        for f in range(4):
            base = XT_OFF + (kc * 4 + f) * 128
            nc.tensor.matmul(t[:, :256], bsb[:, base:base + 128],
                             bsb[:, WV_OFF + f * 256:WV_OFF + (f + 1) * 256],
                             start=(f == 0), stop=(f == 3))
        va = vall[:, kc, :].rearrange("p (h c) -> p h c", c=65)
        nc.vector.tensor_copy(va[:, :, 0:64],
                              t[:, :256].rearrange("p (h c) -> p h c", c=64))

    def kq_proj(cc, qoff, qlen):
        t = pj_tile()
        nparts = len(_xt_moving(bsb, XT_OFF, SP, 0, qoff, qlen))
        for pi in range(nparts):
            for f in range(4):
                if cc < 2:
                    woff = WQ_OFF + f * 256 + cc * 128
                else:
                    woff = WK_OFF + f * 256 + (cc - 2) * 128
                loff, llen, ap = _xt_moving(bsb, XT_OFF, SP, f, qoff, qlen)[pi]
                nc.tensor.matmul(t[:, loff:loff + llen], bsb[:, woff:woff + 128],
                                 ap, start=(f == 0), stop=(f == 3))
        if cc < 2:  # q: add etype_emb (which includes bq)
            es = bsb[:, ETE_OFF + cc * SP + qoff:ETE_OFF + cc * SP + qoff + qlen]
            nc.vector.tensor_add(qk[:, cc, qoff:qoff + qlen], t[:, :qlen], es)
        else:
            nc.vector.tensor_copy(qk[:, cc, qoff:qoff + qlen], t[:, :qlen])

    if abl == "proj":
        for kc in range(NKC):
            v_proj(kc)
        for cc in (2, 3):
            for qoff, qlen in KCH:
                kq_proj(cc, qoff, qlen)
        for cc in (0, 1):
            for qoff, qlen in QPCH:
                kq_proj(cc, qoff, qlen)
        return

    # ---- attention steps; kc == -1 marks a head-pair's merged straggler step
    steps = []
    for hp in (0, 1):
        for qc in range(NQCM):
            steps += [(hp, qc, kc) for kc in sigma]
        if MERGE:
            steps.append((hp, 2, -1))
    DEPTH = 2

    # ---- filler schedule: distribute projection work into the attention
    # steps so the (in-order) PE never idles while ACT churns exps.
    # HARD CONSTRAINT: a filler consumed by scores of step j must be emitted
    # at slot <= j - DEPTH - 1 (the scores of step j are emitted during
    # iteration j - DEPTH, before that iteration's fillers); one consumed by
    # PV of step j needs slot <= j - 1. Earlier emission = earlier PE slot,
    # so also keep fillers no earlier than their xT DMA group can land.
    def grp_of(qoff, qlen):
        return ((qoff + qlen - 1) // 128) // 4

    prefix, fillers = [], {}

    def sched(idx, thunk):
        if idx is None or idx < 0 or abl == "nofill":
            prefix.append(thunk)
        else:
            fillers.setdefault(idx, []).append(thunk)

    STEPS_PER_HP = len(steps) // 2

    def first_step_with_kc(kc):  # first step index whose scores touch kc
        return sigma.index(kc)

    def first_step_with_q(hp, qoff):  # first step reading qT[cc=hp] at qoff
        for i, (shp, sqc, skc) in enumerate(steps):
            if shp == hp and QC[sqc][0] <= qoff < QC[sqc][0] + QC[sqc][1]:
                return i
        return len(steps)

    # v(kc): consumed by PV at step idx sigma.index(kc).
    for kc in range(NKC):
        j = first_step_with_kc(kc)
        sched(None if j < 4 else j - 1, lambda kc=kc: v_proj(kc))
    # kT chunks: cc=2 feeds hp0 (deadline-tight), cc=3 feeds hp1 (slack).
    for qoff, qlen in KCH:
        g = grp_of(qoff, qlen)
        j2 = min(first_step_with_kc(kc) for kc in range(qoff // 128, (qoff + qlen) // 128))
        ddl2 = j2 - DEPTH - 1
        sched(None if g == 0 else ddl2, lambda o=qoff, l=qlen: kq_proj(2, o, l))
        ddl3 = STEPS_PER_HP + j2 - DEPTH - 1
        sched(min(ddl3, max(3, 2 * g + 8)), lambda o=qoff, l=qlen: kq_proj(3, o, l))
    # qT chunks: cc=0 feeds hp0, cc=1 feeds hp1.
    for qoff, qlen in QPCH:
        g = grp_of(qoff, qlen)
        ddl0 = first_step_with_q(0, qoff) - DEPTH - 1
        sched(None if g == 0 else min(ddl0, max(5, 2 * g + 9)),
              lambda o=qoff, l=qlen: kq_proj(0, o, l))
        ddl1 = first_step_with_q(1, qoff) - DEPTH - 1
        sched(min(ddl1, max(6, 2 * g + 10)) if g or STEPS_PER_HP > 8 else None,
              lambda o=qoff, l=qlen: kq_proj(1, o, l))

    def scores_mm(step, t):
        hp, qc, kc = step
        if kc < 0:  # merged straggler step: all NKC key chunks, thin q tail
            qoff, SWl = QC[2]
            for s in range(2):
                cb = s * 64
                rhs = qk[cb:cb + 64, hp, qoff:qoff + SWl]
                for k2 in range(NKC):
                    lhsT = qk[cb:cb + 64, 2 + hp, k2 * 128:(k2 + 1) * 128]
                    nc.tensor.matmul(t[:, s, k2 * SWl:(k2 + 1) * SWl], lhsT, rhs,
                                     start=True, stop=True)
            return
        qoff, qlen = QC[qc]
        for s in range(2):
            cb = s * 64
            lhsT = qk[cb:cb + 64, 2 + hp, kc * 128:(kc + 1) * 128]
            rhs = qk[cb:cb + 64, hp, qoff:qoff + qlen]
            nc.tensor.matmul(t[:, s, :qlen], lhsT, rhs, start=True, stop=True)

    hts = {}
    hpt = {}
    DEPTH = 2
    spq = []
    for th in prefix:
        th()
    for d in range(min(DEPTH, len(steps))):
        t = sp_tile()
        scores_mm(steps[d], t)
        spq.append(t)
    last_kc = sigma[-1]
    for i, step in enumerate(steps):
        hp, qc, kc = step
        qoff, qlen = QC[qc]
        sp_cur = spq.pop(0)
        if i + DEPTH < len(steps):
            t = sp_tile()
            scores_mm(steps[i + DEPTH], t)
            spq.append(t)
        if pad:  # dummy PE work: keeps tensor-engine duty above ACT's so the
            # HAM clock gate never re-throttles the PE to 1.2 GHz
            pt = ps.tile([128, 512], F32, tag="pj", bufs=2, name="pj")
            nc.tensor.matmul(pt[:, :pad], zt[:, :128], zt[:, 128:128 + pad],
                             start=True, stop=True)
        att = attp.tile([128, 2, 512], BF, tag="att", name="att")
        if abl == "noexp":  # att never written by ACT: one tiny write so the
            # tile allocates; PV streams garbage (timing-only ablation)
            nc.vector.memset(att[:, :, 0:1], 0.0)
        if kc < 0:  # merged straggler step: one exp + PV chain per head
            SWl = qlen
            for s in range(2):
                h = 2 * hp + s
                if abl != "noexp":
                    nc.scalar.activation(att[:, s, :NKC * SWl], sp_cur[:, s, :NKC * SWl],
                                         mybir.ActivationFunctionType.Exp, scale=0.125)
                if abl == "nopv":
                    continue
                hpt[s] = ps.tile([65, 512], F32, tag="hp", bufs=2, name="hp")
                for k2 in range(NKC):
                    nc.tensor.matmul(hpt[s][:, :SWl], vall[:, k2, h * 65:(h + 1) * 65],
                                     att[:, s, k2 * SWl:(k2 + 1) * SWl],
                                     start=(k2 == 0), stop=(k2 == NKC - 1))
                nc.vector.tensor_copy(hts[h][:, qoff:qoff + SWl], hpt[s][:, :SWl])
                nc.sync.dma_start(out=outp[h, :, :SPL], in_=hts[h][:, :SPL])
            for th in fillers.get(i, []):
                th()
            continue
        if abl != "noexp":
            if qlen == 512:  # contiguous across both head slots: one flat AP
                nc.scalar.activation(att[:].rearrange("p a b -> p (a b)"),
                                     sp_cur[:].rearrange("p a b -> p (a b)"),
                                     mybir.ActivationFunctionType.Exp, scale=0.125)
            else:
                nc.scalar.activation(att[:, :, :qlen], sp_cur[:, :, :qlen],
                                     mybir.ActivationFunctionType.Exp, scale=0.125)
        if abl != "nopv":
            for s in range(2):
                h = 2 * hp + s
                if qc == 0 and kc == 0:
                    hts[h] = hsb_pool.tile([65, SP], F32, tag="hts", name="hts")
                if kc == 0:
                    hpt[s] = ps.tile([65, 512], F32, tag="hp", bufs=2, name="hp")
                nc.tensor.matmul(hpt[s][:, :qlen], vall[:, kc, h * 65:(h + 1) * 65],
                                 att[:, s, :qlen], start=(kc == 0), stop=(kc == last_kc))
            if kc == last_kc:
                for s in range(2):
                    h = 2 * hp + s
                    nc.vector.tensor_copy(hts[h][:, qoff:qoff + qlen], hpt[s][:, :qlen])
                if qc == NQCM - 1 and not MERGE:  # ship hT (+den row) on the
                    # sync ring (idle by now; the ACT queue must stay clear)
                    for s in range(2):
                        h = 2 * hp + s
                        nc.sync.dma_start(out=outp[h, :, :SPL], in_=hts[h][:, :SPL])
        for th in fillers.get(i, []):
            th()
    # any fillers scheduled past the end
    for i in sorted(fillers):
        if i >= len(steps):
            for th in fillers[i]:
                th()


def _prep_core(core, SP, x, etype_emb, mask, Wq, bq, Wk, bk, Wv, bv):
    NKC = SP // 128
    (WV_OFF, WK_OFF, WQ_OFF, XT_OFF, ETE_OFF, BK_OFF, BV_OFF, ONES_OFF,
     LIVE2_OFF, COLS) = _offsets(SP)
    b, hg = core // 2, core % 2
    c0 = hg * CPC
    idx = np.where(mask[b] == 1)[0]
    Su = len(idx)

    blob = np.zeros((128, COLS), np.float32)
    xs = np.zeros((SP, F), np.float32)
    xs[:Su] = x[b][idx]
    xT = xs.T
    xtb = xT.reshape(4, 128, NKC, 128).transpose(1, 2, 0, 3).reshape(128, NKC * 512)
    blob[:, XT_OFF:XT_OFF + 4 * SP] = xtb
    for f in range(4):
        blob[:, WV_OFF + f * 256:WV_OFF + (f + 1) * 256] = Wv[f * 128:(f + 1) * 128, c0:c0 + CPC]
        blob[:, WK_OFF + f * 256:WK_OFF + (f + 1) * 256] = Wk[f * 128:(f + 1) * 128, c0:c0 + CPC]
        blob[:, WQ_OFF + f * 256:WQ_OFF + (f + 1) * 256] = Wq[f * 128:(f + 1) * 128, c0:c0 + CPC]
    et = np.zeros((SP, CPC), np.float32)
    et[:Su] = etype_emb[b][idx][:, c0:c0 + CPC] + bq[c0:c0 + CPC]
    etT = et.T
    blob[:, ETE_OFF:ETE_OFF + SP] = etT[:128]
    blob[:, ETE_OFF + SP:ETE_OFF + 2 * SP] = etT[128:]
    blob[0, BK_OFF:BK_OFF + CPC] = bk[c0:c0 + CPC]
    blob[0, BV_OFF:BV_OFF + CPC] = bv[c0:c0 + CPC]
    blob[0, ONES_OFF:ONES_OFF + SP] = 1.0
    pos = np.arange(128)[:, None] + 128 * np.arange(NKC)[None, :]
    live = (pos < Su).astype(np.float32)          # [128, NKC]
    blob[:, LIVE2_OFF:LIVE2_OFF + NKC * HPC] = np.repeat(live, HPC, axis=1)

    return {"blob": blob}, idx


def kernel(x, etype_emb, mask, Wq, bq, Wk, bk, Wv, bv):
    global LAST_RESULT
    x = np.asarray(x, np.float32)
    etype_emb = np.asarray(etype_emb, np.float32)
    mask = np.asarray(mask)
    Wq, bq = np.asarray(Wq, np.float32), np.asarray(bq, np.float32)
    Wk, bk = np.asarray(Wk, np.float32), np.asarray(bk, np.float32)
    Wv, bv = np.asarray(Wv, np.float32), np.asarray(bv, np.float32)

    counts = [int((mask[b] == 1).sum()) for b in range(B)]
    SPL = max(2, max(counts))
    SPL += SPL % 2  # fp32r matmuls with 128 contraction rows need even N
    SP = max(128, ((SPL + 127) // 128) * 128)

    nc = _build(SP, SPL=SPL)
    in_maps, idxs = [], []
    for core in range(NCORES):
        m, idx = _prep_core(core, SP, x, etype_emb, mask, Wq, bq, Wk, bk, Wv, bv)
        in_maps.append(m)
        idxs.append(idx)

    # The NTFF trace path needs antenv.axon_hooks, which this container does
    # not ship; make sure a stray BASS_TRACE=1 cannot route us into it.
    os.environ.setdefault("BASS_NEVER_TRACE", "1")
    res = run_bass_kernel_spmd(nc, in_maps, list(range(NCORES)))
    LAST_RESULT = res

    out = np.zeros((B, S, E), np.float32)
    for core in range(NCORES):
        b, hg = core // 2, core % 2
        idx = idxs[core]
        if not len(idx):
            continue
        shard = res.results[core]["out"]  # [HPC, 65, SP]: hT rows + denominator
        for h in range(HPC):
            num = shard[h, :64, :len(idx)]
            den = shard[h, 64, :len(idx)]
            bvh = bv[hg * CPC + h * 64:hg * CPC + (h + 1) * 64]
            out[b][idx, hg * CPC + h * 64:hg * CPC + (h + 1) * 64] = (num / den).T + bvh
    return out
